# revision 8
# baseline (speedup 1.0000x reference)
"""Trainium2 Bass kernel for the patch-Mamba time-series model.

Sharding: data-parallel over the B*M=112 flattened batch axis across 8 cores
(14 sequences per core). All weights replicated.

Per-core layout: feature-major activations [feature-partitions, (seq,token) free].
The selective scan runs on the Vector engine via tensor_tensor_scan with the
recurrence chained along the free dim (sequence boundaries reset by forcing
dA=0 at t=0 of each sequence). dA = exp(A_s * delta) is produced by the Scalar
engine (one exp pass per state index, exploiting that A is d-independent).
"""

import sys

sys.path.insert(0, "/opt/trn_rl_repo")

import numpy as np
import ml_dtypes

import concourse.bass as bass
import concourse.mybir as mybir
import concourse.tile as tile
from concourse import bass_utils

F32 = mybir.dt.float32
BF16 = mybir.dt.bfloat16
AL = mybir.AluOpType
AF = mybir.ActivationFunctionType

# dims
B, M, SEQ = 16, 7, 512
PATCH, STRIDE, NPATCH = 16, 8, 64
D_MODEL, N_LAYERS, PRED = 256, 2, 96
D_INNER, D_STATE, DT_RANK, D_CONV = 512, 16, 16, 4
EPS = 1e-5
NCORES = 8
NSEQ = (B * M) // NCORES          # 14 sequences per core
NT = NSEQ * NPATCH                # 896 tokens per core
NH = 2                            # n-halves for matmul N<=512
NTH = NT // NH                    # 448
XPD = DT_RANK + 2 * D_STATE       # 48
KHEAD = (NPATCH * D_MODEL) // 128  # 128 k-blocks for the head

_CACHE = {}


def _legalize_pe_waits(nc):
    """walrus codegen accepts only ONE sync-wait on a PE Matmult (S3_LW
    struct); hoist extra waits onto standalone EventSemaphore carriers
    inserted immediately before the offending instruction."""
    nid = [0]
    for f in nc.m.functions:
        for blk in f.blocks:
            out = []
            changed = False
            for i in blk.instructions:
                si = getattr(i, "sync_info", None)
                tn = type(i).__name__
                eng = getattr(i, "engine", None)
                if (si is not None and si.on_wait is not None
                        and len(si.on_wait) > 1
                        and tn != "InstEventSemaphore"
                        and eng is not None
                        and eng != mybir.EngineType.Unassigned):
                    waits = list(si.on_wait)
                    for w in waits[:-1]:
                        ev = mybir.InstEventSemaphore(
                            name=f"WSPLIT-{nid[0]}", ins=[], outs=[])
                        nid[0] += 1
                        ev.engine = eng
                        ev.sync_info = mybir.SyncInfo(on_wait=[w], on_update=[])
                        out.append(ev)
                    i.sync_info = mybir.SyncInfo(
                        on_wait=[waits[-1]], on_update=list(si.on_update))
                    changed = True
                out.append(i)
            if changed:
                blk.instructions = out


def _build(a_scales):
    """Emit the per-core program. a_scales[l][s] = A[l, d, s] (d-independent)."""
    nc = bass.Bass("TRN2", target_bir_lowering=False)

    # ---- dram inputs ----
    def din(name, shape, dt=F32):
        return nc.dram_tensor(name, shape, dt, kind="ExternalInput")

    xpatch = din("xpatch", [PATCH, NT])                  # per-core unfolded patches
    posW = din("posW", [PATCH, D_MODEL])
    posb = din("posb", [128, 2])                          # col = dm half
    posembT = din("posembT", [128, 2 * NPATCH])           # col = half*64+t
    rmsw = din("rmsw", [128, N_LAYERS * 2])               # col = l*2+half
    inW = din("inW", [128, N_LAYERS * 2 * 2 * D_INNER], BF16)   # (l,kb) major
    convw = din("convw", [128, N_LAYERS * 16])            # col = l*16+db*4+k
    convb = din("convb", [128, N_LAYERS * 4])             # col = l*4+db
    xpW = din("xpW", [128, N_LAYERS * 4 * XPD], BF16)     # (l,kb) major
    dtW = din("dtW", [DT_RANK, N_LAYERS * D_INNER], BF16)  # col = l*512+j
    dtb = din("dtb", [128, N_LAYERS * 4])
    Dskip = din("Dskip", [128, N_LAYERS * 4])
    outW = din("outW", [128, N_LAYERS * 4 * D_MODEL], BF16)  # (l,kb) major
    lng = din("lng", [128, 2])
    lnb = din("lnb", [128, 2])
    headW = din("headW", [128, KHEAD * PRED])             # col = kb*96+j, f32
    headb = din("headb", [NSEQ, PRED])                    # host-replicated rows
    ones_in = din("ones", [128, 128])
    ident_in = din("ident", [128, 128], BF16)
    epsc = din("epsc", [128, 1])

    yout = nc.dram_tensor("yout", [NSEQ, PRED], F32, kind="ExternalOutput")

    with tile.TileContext(nc) as tc:
        import contextlib

        ctx = contextlib.ExitStack()
        with ctx:
            cp = ctx.enter_context(tc.tile_pool(name="consts", bufs=1))
            wp = ctx.enter_context(tc.tile_pool(name="work", bufs=1))
            ep = ctx.enter_context(tc.tile_pool(name="escan", bufs=2))
            pp = ctx.enter_context(tc.tile_pool(name="psum", bufs=2, space="PSUM"))
            pps = ctx.enter_context(tc.tile_pool(name="psum_s", bufs=1, space="PSUM"))
            yp = ctx.enter_context(tc.tile_pool(name="psum_y", bufs=2, space="PSUM"))
            dp = ctx.enter_context(tc.tile_pool(name="dram", bufs=2, space="DRAM"))

            # ---- load consts ----
            def cload(name, src, shape, dt=F32):
                t = cp.tile(shape, dt, tag=name, name=name)
                nc.sync.dma_start(t[:], src[:])
                return t

            posW_t = cload("posW", posW, [PATCH, D_MODEL])
            posb_t = cload("posb", posb, [128, 2])
            pose_t = cload("posembT", posembT, [128, 2 * NPATCH])
            rmsw_t = cload("rmsw", rmsw, [128, N_LAYERS * 2])
            inW_t = cload("inW", inW, [128, N_LAYERS * 2 * 2 * D_INNER], BF16)
            convw_t = cload("convw", convw, [128, N_LAYERS * 16])
            convb_t = cload("convb", convb, [128, N_LAYERS * 4])
            xpW_t = cload("xpW", xpW, [128, N_LAYERS * 4 * XPD], BF16)
            dtW_t = cload("dtW", dtW, [DT_RANK, N_LAYERS * D_INNER], BF16)
            dtb_t = cload("dtb", dtb, [128, N_LAYERS * 4])
            Dsk_t = cload("Dskip", Dskip, [128, N_LAYERS * 4])
            outW_t = cload("outW", outW, [128, N_LAYERS * 4 * D_MODEL], BF16)
            lng_t = cload("lng", lng, [128, 2])
            lnb_t = cload("lnb", lnb, [128, 2])
            headb_t = cload("headb", headb, [NSEQ, PRED])
            ones_t = cload("ones", ones_in, [128, 128])
            ident_t = cload("ident", ident_in, [128, 128], BF16)
            eps_t = cload("epsc", epsc, [128, 1])

            # patches rhs [16 partitions, (n,t)=896], unfolded host-side
            patches = cp.tile([PATCH, NT], F32, tag="patches", name="patches")
            nc.sync.dma_start(patches[:], xpatch[:])

            def nsl(nh):
                return slice(nh * NTH, (nh + 1) * NTH)

            def bcast_mid(ap2d, count):
                """[P, T] AP -> [P, count, T] with the middle dim broadcast."""
                aps = list(ap2d.ap)
                return bass.AP(ap2d.tensor, ap2d.offset,
                               [list(aps[0]), [0, count], list(aps[1])])

            # ---- positional encoding: h = patches @ posW + posb + posembT ----
            h = [wp.tile([128, NT], F32, tag=f"h{b}", name=f"h{b}") for b in range(2)]
            for b in range(2):
                for nh in range(NH):
                    ps = pp.tile([128, NTH], F32, tag="mm", name="mm")
                    nc.tensor.matmul(
                        ps[:], posW_t[:, b * 128:(b + 1) * 128],
                        patches[:, nsl(nh)], start=True, stop=True,
                    )
                    # h = psum + posb (per-partition) + posemb (bcast over n)
                    pe = bcast_mid(pose_t[:, b * NPATCH:(b + 1) * NPATCH], NSEQ // NH)
                    nc.vector.scalar_tensor_tensor(
                        h[b][:, nsl(nh)].rearrange("p (n t) -> p n t", t=NPATCH),
                        ps[:].rearrange("p (n t) -> p n t", t=NPATCH),
                        posb_t[:, b:b + 1],
                        pe,
                        AL.add, AL.add,
                    )

            stat_d = None

            def colnorm_rsqrt(rhs_tiles, scale, tag):
                """Column variance-ish: rs_rep[p,c] = 1/sqrt(scale*sum_p(rhs) + EPS).

                rhs_tiles: two [128, NT] f32 tiles whose partition-sums to take.
                Returns ([128,NT] sum_rep f32 tile, [128,NT] rs_rep f32 tile).
                """
                sum_rep = wp.tile([128, NT], F32, tag="sumrep", name="sumrep")
                sd_rep = wp.tile([128, NT], F32, tag="sdrep", name="sdrep")
                for nh in range(NH):
                    ps = pps.tile([128, NTH], F32, tag="red", name="red")
                    nc.tensor.matmul(ps[:], ones_t[:], rhs_tiles[0][:, nsl(nh)],
                                     start=True, stop=False)
                    nc.tensor.matmul(ps[:], ones_t[:], rhs_tiles[1][:, nsl(nh)],
                                     start=False, stop=True)
                    nc.scalar.copy(sum_rep[:, nsl(nh)], ps[:])
                    nc.scalar.activation(sd_rep[:, nsl(nh)], ps[:], AF.Sqrt,
                                         bias=eps_t[:, 0:1], scale=scale)
                # reciprocal via [128,7] round trip
                d1 = dp.tile([1, NT], F32, tag="statd", name="statd")
                nc.sync.dma_start(d1[:], sd_rep[0:1, :])
                small = wp.tile([128, NT // 128], F32, tag="recip_small", name="recip_small")
                dsrc = bass.AP(d1[:].tensor, d1[:].offset, [[1, 128], [128, NT // 128]])
                nc.sync.dma_start(small[:], dsrc)
                nc.vector.reciprocal(small[:], small[:])
                d2 = dp.tile([1, NT], F32, tag="statd2", name="statd2")
                nc.sync.dma_start(
                    bass.AP(d2[:].tensor, d2[:].offset, [[1, 128], [128, NT // 128]]),
                    small[:],
                )
                rs_rep = wp.tile([128, NT], F32, tag="rsrep", name="rsrep")
                nc.sync.dma_start(rs_rep[:], d2[:].broadcast_to([128, NT]))
                return sum_rep, rs_rep

            # =================== layers ===================
            for l in range(N_LAYERS):
                # ---- RMSNorm -> xn (bf16) ----
                hsq = [wp.tile([128, NT], F32, tag=f"hsq{b}", name=f"hsq{b}") for b in range(2)]
                for b in range(2):
                    nc.scalar.square(hsq[b][:], h[b][:])
                _, rs_rep = colnorm_rsqrt(hsq, 1.0 / D_MODEL, f"rms{l}")
                xn = [wp.tile([128, NT], BF16, tag=f"xn{b}", name=f"xn{b}") for b in range(2)]
                for b in range(2):
                    nc.vector.scalar_tensor_tensor(
                        xn[b][:], h[b][:], rmsw_t[:, l * 2 + b:l * 2 + b + 1],
                        rs_rep[:], AL.mult, AL.mult,
                    )

                # ---- in_proj -> v (pre-conv xi), sz (silu(z)) ----
                v = [wp.tile([128, NT], BF16, tag=f"v{db}", name=f"v{db}") for db in range(4)]
                sz = [wp.tile([128, NT], BF16, tag=f"sz{db}", name=f"sz{db}") for db in range(4)]
                for mb in range(8):
                    for nh in range(NH):
                        ps = pp.tile([128, NTH], F32, tag="mm", name="mm")
                        for kb in range(2):
                            w0 = (l * 2 + kb) * (2 * D_INNER) + mb * 128
                            nc.tensor.matmul(
                                ps[:], inW_t[:, w0:w0 + 128], xn[kb][:, nsl(nh)],
                                start=(kb == 0), stop=(kb == 1),
                            )
                        if mb < 4:
                            nc.scalar.copy(v[mb][:, nsl(nh)], ps[:])
                        else:
                            nc.scalar.activation(sz[mb - 4][:, nsl(nh)], ps[:], AF.Silu)

                # ---- causal depthwise conv + silu -> u ----
                u = [wp.tile([128, NT], BF16, tag=f"u{db}", name=f"u{db}") for db in range(4)]
                ca = [wp.tile([128, NT], BF16, tag=f"ca{db}", name=f"ca{db}") for db in range(4)]
                for db in range(4):
                    c0 = l * 16 + db * 4
                    nc.vector.tensor_scalar_mul(ca[db][:], v[db][:],
                                                convw_t[:, c0 + 3:c0 + 4])
                    cav = ca[db][:].rearrange("p (n t) -> p n t", t=NPATCH)
                    vv = v[db][:].rearrange("p (n t) -> p n t", t=NPATCH)
                    for k in range(1, D_CONV):
                        nc.vector.scalar_tensor_tensor(
                            cav[:, :, k:], vv[:, :, :NPATCH - k],
                            convw_t[:, c0 + 3 - k:c0 + 4 - k],
                            cav[:, :, k:], AL.mult, AL.add,
                        )
                    nc.scalar.activation(u[db][:], ca[db][:], AF.Silu,
                                         bias=convb_t[:, l * 4 + db:l * 4 + db + 1])

                # ---- x_proj -> bc = [dt; B; C] feature-major [48, NT] bf16 ----
                bc = wp.tile([XPD, NT], BF16, tag="bc", name="bc")
                for nh in range(NH):
                    ps = pp.tile([XPD, NTH], F32, tag="mm48", name="mm48", bufs=1)
                    for kb in range(4):
                        w0 = (l * 4 + kb) * XPD
                        nc.tensor.matmul(
                            ps[:], xpW_t[:, w0:w0 + XPD], u[kb][:, nsl(nh)],
                            start=(kb == 0), stop=(kb == 3),
                        )
                    nc.scalar.copy(bc[:, nsl(nh)], ps[:])

                # ---- delta = softplus(dt @ dtW + dtb) f32 [512, NT] ----
                delta = [wp.tile([128, NT], F32, tag=f"delta{db}", name=f"delta{db}") for db in range(4)]
                sptmp = wp.tile([128, NT], F32, tag="hsq0", name="sptmp")
                for db in range(4):
                    for nh in range(NH):
                        ps = pp.tile([128, NTH], F32, tag="mm", name="mm")
                        w0 = l * D_INNER + db * 128
                        nc.tensor.matmul(
                            ps[:], dtW_t[:, w0:w0 + 128], bc[0:DT_RANK, nsl(nh)],
                            start=True, stop=True,
                        )
                        # softplus(x) = ln(1 + exp(x)); Softplus has no ACT table
                        nc.scalar.activation(
                            sptmp[:, nsl(nh)], ps[:], AF.Exp,
                            bias=dtb_t[:, l * 4 + db:l * 4 + db + 1],
                        )
                        nc.scalar.activation(
                            delta[db][:, nsl(nh)], sptmp[:, nsl(nh)], AF.Ln,
                            bias=1.0,
                        )

                # ---- u' = delta * u (bf16) ----
                up = [wp.tile([128, NT], BF16, tag=f"up{db}", name=f"up{db}") for db in range(4)]
                for db in range(4):
                    nc.vector.tensor_mul(up[db][:], delta[db][:], u[db][:])

                # force dA=0 at sequence starts: delta[:, n*64] = large
                for db in range(4):
                    dv = delta[db][:].rearrange("p (n t) -> p n t", t=NPATCH)
                    nc.vector.memset(dv[:, :, 0:1], 1.0e30)

                # B,C rows to DRAM for partition-replication
                bc_d = dp.tile([2 * D_STATE, NT], BF16, tag="bc_d", name="bc_d")
                nc.sync.dma_start(bc_d[:], bc[DT_RANK:XPD, :])

                # ---- selective scan over 16 states ----
                # Two db-pair passes; y accumulated in PSUM via identity
                # matmuls on the (otherwise idle) PE. Some elementwise mults
                # offloaded to GpSimd to unload the Vector engine.
                yf = [wp.tile([128, NT], BF16, tag=f"v{db}", name=f"yf{db}") for db in range(4)]
                for hp in range(2):
                    dbs = (2 * hp, 2 * hp + 1)
                    ya = {db: yp.tile([128, 960], F32, tag="yacc", name=f"ya{db}",
                                      padded_shape=[128, 1024])
                          for db in dbs}
                    for s in range(D_STATE):
                        brep = ep.tile([128, NT], BF16, tag="brep", name="brep")
                        crep = ep.tile([128, NT], BF16, tag="crep", name="crep")
                        nc.sync.dma_start(
                            brep[:], bc_d[s:s + 1, :].broadcast_to([128, NT]))
                        nc.sync.dma_start(
                            crep[:],
                            bc_d[D_STATE + s:D_STATE + s + 1, :].broadcast_to([128, NT]))
                        for db in dbs:
                            dA = ep.tile([128, NT], BF16, tag=f"dA{db}", name=f"dA{db}")
                            nc.scalar.activation(dA[:], delta[db][:], AF.Exp,
                                                 scale=float(a_scales[l][s]))
                            dBx = ep.tile([128, NT], BF16, tag=f"dBx{db}", name=f"dBx{db}")
                            if s % 4 == 2:
                                nc.gpsimd.tensor_mul(dBx[:], up[db][:], brep[:])
                            else:
                                nc.vector.tensor_mul(dBx[:], up[db][:], brep[:])
                            hs = ep.tile([128, NT], BF16, tag=f"hs{db}", name=f"hs{db}")
                            nc.vector.tensor_tensor_scan(
                                hs[:], dA[:], dBx[:], 0.0, AL.mult, AL.add)
                            ch = ep.tile([128, NT], BF16, tag=f"dA{db}", name=f"ch{db}")
                            if s % 4 != 0:
                                nc.gpsimd.tensor_mul(ch[:], hs[:], crep[:])
                            else:
                                nc.vector.tensor_mul(ch[:], hs[:], crep[:])
                            for nh in range(NH):
                                nc.tensor.matmul(
                                    ya[db][:, nh * 512:nh * 512 + NTH],
                                    ident_t[:], ch[:, nsl(nh)],
                                    start=(s == 0), stop=(s == D_STATE - 1),
                                )

                    # ---- y = (u*Dskip + yacc) * sz ----
                    for db in dbs:
                        for nh in range(NH):
                            nc.vector.scalar_tensor_tensor(
                                yf[db][:, nsl(nh)], u[db][:, nsl(nh)],
                                Dsk_t[:, l * 4 + db:l * 4 + db + 1],
                                ya[db][:, nh * 512:nh * 512 + NTH],
                                AL.mult, AL.add,
                            )
                        nc.vector.tensor_mul(yf[db][:], yf[db][:], sz[db][:])

                # ---- out_proj + residual into h ----
                for mb in range(2):
                    for nh in range(NH):
                        ps = pp.tile([128, NTH], F32, tag="mm", name="mm")
                        for kb in range(4):
                            w0 = (l * 4 + kb) * D_MODEL + mb * 128
                            nc.tensor.matmul(
                                ps[:], outW_t[:, w0:w0 + 128], yf[kb][:, nsl(nh)],
                                start=(kb == 0), stop=(kb == 3),
                            )
                        nc.vector.tensor_add(h[mb][:, nsl(nh)], h[mb][:, nsl(nh)], ps[:])

            # =================== final LayerNorm ===================
            hsq = [wp.tile([128, NT], F32, tag=f"hsq{b}", name=f"hsq{b}") for b in range(2)]
            for b in range(2):
                nc.scalar.square(hsq[b][:], h[b][:])
            msq_rep, _ = colnorm_rsqrt(hsq, 1.0 / D_MODEL, "lnsq")
            mu_rep = wp.tile([128, NT], F32, tag="hsq1", name="murep")
            for nh in range(NH):
                ps = pps.tile([128, NTH], F32, tag="red", name="red")
                nc.tensor.matmul(ps[:], ones_t[:], h[0][:, nsl(nh)], start=True, stop=False)
                nc.tensor.matmul(ps[:], ones_t[:], h[1][:, nsl(nh)], start=False, stop=True)
                nc.scalar.mul(mu_rep[:, nsl(nh)], ps[:], 1.0 / D_MODEL)
            # var = msq/256 - mu^2   (msq_rep holds sum; scale later)
            var = wp.tile([128, NT], F32, tag="hsq0", name="var")
            nc.vector.tensor_mul(var[:], mu_rep[:], mu_rep[:])
            sd_rep = wp.tile([128, NT], F32, tag="sdrep", name="lnsd")
            # sd = sqrt(msq/256 - mu^2 + eps): compute msq/256 - mu2 first
            nc.scalar.mul(msq_rep[:], msq_rep[:], 1.0 / D_MODEL)
            nc.vector.tensor_sub(var[:], msq_rep[:], var[:])
            nc.scalar.activation(sd_rep[:], var[:], AF.Sqrt, bias=eps_t[:, 0:1])
            d1 = dp.tile([1, NT], F32, tag="statd", name="statd")
            nc.sync.dma_start(d1[:], sd_rep[0:1, :])
            small = wp.tile([128, NT // 128], F32, tag="recip_small", name="recip_small")
            nc.sync.dma_start(
                small[:], bass.AP(d1[:].tensor, d1[:].offset, [[1, 128], [128, NT // 128]]))
            nc.vector.reciprocal(small[:], small[:])
            d2 = dp.tile([1, NT], F32, tag="statd2", name="statd2")
            nc.sync.dma_start(
                bass.AP(d2[:].tensor, d2[:].offset, [[1, 128], [128, NT // 128]]), small[:])
            rs_rep = wp.tile([128, NT], F32, tag="rsrep", name="lnrs")
            nc.sync.dma_start(rs_rep[:], d2[:].broadcast_to([128, NT]))

            hn = [wp.tile([128, NT], F32, tag=f"delta{b}", name=f"hn{b}") for b in range(2)]
            for b in range(2):
                nc.vector.tensor_sub(hn[b][:], h[b][:], mu_rep[:])
                nc.vector.tensor_mul(hn[b][:], hn[b][:], rs_rep[:])
                nc.vector.tensor_scalar(
                    hn[b][:], hn[b][:], lng_t[:, b:b + 1], lnb_t[:, b:b + 1],
                    AL.mult, AL.add,
                )

            # =================== head (fp32) ===================
            psh_full = pps.tile([128, NTH], F32, tag="red", name="head")
            psh = psh_full[0:NSEQ, 0:PRED]
            HCH = 32  # kb-blocks per headW chunk
            for hc in range(KHEAD // HCH):
                headW_t = cp.tile([128, HCH * PRED], F32, tag="headWc",
                                  name="headWc", bufs=2)
                nc.sync.dma_start(
                    headW_t[:], headW[:, hc * HCH * PRED:(hc + 1) * HCH * PRED])
                for j in range(HCH):
                    kb = hc * HCH + j
                    b = kb % 2
                    t = kb // 2
                    lhsT = bass.AP(
                        hn[b][:].tensor, hn[b][:].offset + t,
                        [[hn[b][:].ap[0][0], 128], [NPATCH, NSEQ]],
                    )
                    nc.tensor.matmul(
                        psh[:], lhsT, headW_t[:, j * PRED:(j + 1) * PRED],
                        start=(kb == 0), stop=(kb == KHEAD - 1),
                    )
            yo = wp.tile([NSEQ, PRED], F32, tag="yo", name="yo")
            nc.vector.tensor_add(yo[:], psh[:], headb_t[:])
            nc.sync.dma_start(yout[:], yo[:])

    _legalize_pe_waits(nc)
    return nc


def _prep_shared(inp):
    """Build the shared (replicated) input arrays from the full inputs."""
    f32 = np.float32
    bf = ml_dtypes.bfloat16
    out = {}
    out["posW"] = np.asarray(inp["pos_W"], f32)
    pb = np.zeros((128, 2), f32)
    pb[:, 0] = np.asarray(inp["pos_b"], f32)[:128]
    pb[:, 1] = np.asarray(inp["pos_b"], f32)[128:]
    out["posb"] = pb
    pe = np.asarray(inp["pos_emb"], f32)  # [64, 256]
    pet = np.zeros((128, 2 * NPATCH), f32)
    pet[:, :NPATCH] = pe[:, :128].T
    pet[:, NPATCH:] = pe[:, 128:].T
    out["posembT"] = pet
    rw = np.zeros((128, N_LAYERS * 2), f32)
    for l in range(N_LAYERS):
        rwl = np.asarray(inp["rms_w"], f32)[l]
        rw[:, l * 2] = rwl[:128]
        rw[:, l * 2 + 1] = rwl[128:]
    out["rmsw"] = rw
    iw = np.zeros((128, N_LAYERS * 2 * 2 * D_INNER), bf)
    for l in range(N_LAYERS):
        w = np.asarray(inp["in_proj_W"], f32)[l]  # [256, 1024]
        for kb in range(2):
            iw[:, (l * 2 + kb) * 2 * D_INNER:(l * 2 + kb + 1) * 2 * D_INNER] = \
                w[kb * 128:(kb + 1) * 128, :].astype(bf)
    out["inW"] = iw
    cw = np.zeros((128, N_LAYERS * 16), f32)
    cb = np.zeros((128, N_LAYERS * 4), f32)
    dtb_ = np.zeros((128, N_LAYERS * 4), f32)
    dsk = np.zeros((128, N_LAYERS * 4), f32)
    for l in range(N_LAYERS):
        cwl = np.asarray(inp["conv_W"], f32)[l][:, 0, :]  # [512, 4]
        cbl = np.asarray(inp["conv_b"], f32)[l]
        dbl = np.asarray(inp["dt_b"], f32)[l]
        dsl = np.asarray(inp["D_skip"], f32)[l]
        for db in range(4):
            cw[:, l * 16 + db * 4:l * 16 + db * 4 + 4] = cwl[db * 128:(db + 1) * 128, :]
            cb[:, l * 4 + db] = cbl[db * 128:(db + 1) * 128]
            dtb_[:, l * 4 + db] = dbl[db * 128:(db + 1) * 128]
            dsk[:, l * 4 + db] = dsl[db * 128:(db + 1) * 128]
    out["convw"] = cw
    out["convb"] = cb
    out["dtb"] = dtb_
    out["Dskip"] = dsk
    xw = np.zeros((128, N_LAYERS * 4 * XPD), bf)
    for l in range(N_LAYERS):
        w = np.asarray(inp["x_proj_W"], f32)[l]  # [512, 48]
        for kb in range(4):
            xw[:, (l * 4 + kb) * XPD:(l * 4 + kb + 1) * XPD] = \
                w[kb * 128:(kb + 1) * 128, :].astype(bf)
    out["xpW"] = xw
    dw = np.zeros((DT_RANK, N_LAYERS * D_INNER), bf)
    for l in range(N_LAYERS):
        dw[:, l * D_INNER:(l + 1) * D_INNER] = \
            np.asarray(inp["dt_W"], f32)[l].astype(bf)
    out["dtW"] = dw
    ow = np.zeros((128, N_LAYERS * 4 * D_MODEL), bf)
    for l in range(N_LAYERS):
        w = np.asarray(inp["out_proj_W"], f32)[l]  # [512, 256]
        for kb in range(4):
            ow[:, (l * 4 + kb) * D_MODEL:(l * 4 + kb + 1) * D_MODEL] = \
                w[kb * 128:(kb + 1) * 128, :].astype(bf)
    out["outW"] = ow
    lg = np.zeros((128, 2), f32)
    lb = np.zeros((128, 2), f32)
    lg[:, 0] = np.asarray(inp["ln_g"], f32)[:128]
    lg[:, 1] = np.asarray(inp["ln_g"], f32)[128:]
    lb[:, 0] = np.asarray(inp["ln_b"], f32)[:128]
    lb[:, 1] = np.asarray(inp["ln_b"], f32)[128:]
    out["lng"] = lg
    out["lnb"] = lb
    hw = np.asarray(inp["head_W"], f32)  # [16384, 96]
    out["headW"] = np.ascontiguousarray(
        hw.reshape(KHEAD, 128, PRED).transpose(1, 0, 2).reshape(128, KHEAD * PRED))
    out["headb"] = np.broadcast_to(
        np.asarray(inp["head_b"], f32), (NSEQ, PRED)).copy()
    out["ones"] = np.ones((128, 128), f32)
    out["ident"] = np.eye(128, dtype=bf)
    out["epsc"] = np.full((128, 1), EPS, f32)
    return out


def kernel(**inputs):
    x = np.asarray(inputs["x"], np.float32)          # [16, 7, 512]
    A = -np.exp(np.asarray(inputs["A_log"], np.float64))  # [2, 512, 16]
    # A is d-independent by construction; bake per-(l,s) scales as immediates
    a_scales = tuple(tuple(float(A[l, 0, s]) for s in range(D_STATE))
                     for l in range(N_LAYERS))

    key = a_scales
    if key not in _CACHE:
        _CACHE[key] = _build(a_scales)
    nc = _CACHE[key]

    shared = _prep_shared(inputs)
    xf = x.reshape(B * M, SEQ)
    xpad = np.concatenate([xf, np.repeat(xf[:, -1:], STRIDE, axis=1)], axis=1)
    idx = np.arange(NPATCH)[:, None] * STRIDE + np.arange(PATCH)[None, :]
    allpatch = xpad[:, idx]  # [112, 64, 16]

    in_maps = []
    for c in range(NCORES):
        m = dict(shared)
        pc = allpatch[c * NSEQ:(c + 1) * NSEQ]          # [14, 64, 16]
        m["xpatch"] = np.ascontiguousarray(
            pc.reshape(NT, PATCH).T, np.float32)         # [16, 896]
        in_maps.append(m)

    res = bass_utils.run_bass_kernel_spmd(nc, in_maps, core_ids=list(range(NCORES)))
    global LAST_RESULT
    LAST_RESULT = res
    outs = [res.results[c]["yout"] for c in range(NCORES)]
    y = np.concatenate(outs, axis=0)  # [112, 96]
    return y.reshape(B, M, PRED)


if __name__ == "__main__":
    import reference

    inp = {k: np.asarray(v) for k, v in reference.setup_inputs().items()}
    got = kernel(**inp)
    want = np.asarray(reference.reference(**inp))
    err = np.abs(got - want).max() / (np.abs(want).max() + 1e-30)
    print("Relative error:", err)



# revision 22
# speedup vs baseline: 1.2825x; 1.2825x over previous
"""Trainium2 Bass kernel for the patch-Mamba time-series model.

Sharding: data-parallel over the B*M=112 flattened batch axis across 8 cores
(14 sequences per core). All weights replicated.

Per-core layout: feature-major activations [feature-partitions, (seq,token) free].
The selective scan runs on the Vector engine via tensor_tensor_scan with the
recurrence chained along the free dim (sequence boundaries reset by forcing
dA=0 at t=0 of each sequence). dA = exp(A_s * delta) is produced by the Scalar
engine (one exp pass per state index, exploiting that A is d-independent).
"""

import sys

sys.path.insert(0, "/opt/trn_rl_repo")

import numpy as np
import ml_dtypes

import concourse.bass as bass
import concourse.mybir as mybir
import concourse.tile as tile
from concourse import bass_utils

F32 = mybir.dt.float32
BF16 = mybir.dt.bfloat16
AL = mybir.AluOpType
AF = mybir.ActivationFunctionType

# dims
B, M, SEQ = 16, 7, 512
PATCH, STRIDE, NPATCH = 16, 8, 64
D_MODEL, N_LAYERS, PRED = 256, 2, 96
D_INNER, D_STATE, DT_RANK, D_CONV = 512, 16, 16, 4
EPS = 1e-5
NCORES = 8
NSEQ = (B * M) // NCORES          # 14 sequences per core
NT = NSEQ * NPATCH                # 896 tokens per core
NH = 2                            # n-halves for matmul N<=512
NTH = NT // NH                    # 448
XPD = DT_RANK + 2 * D_STATE       # 48
KHEAD = (NPATCH * D_MODEL) // 128  # 128 k-blocks for the head

_CACHE = {}


def _legalize_pe_waits(nc):
    """walrus codegen accepts only ONE sync-wait on a PE Matmult (S3_LW
    struct); hoist extra waits onto standalone EventSemaphore carriers
    inserted immediately before the offending instruction."""
    nid = [0]
    for f in nc.m.functions:
        for blk in f.blocks:
            out = []
            changed = False
            for i in blk.instructions:
                si = getattr(i, "sync_info", None)
                tn = type(i).__name__
                eng = getattr(i, "engine", None)
                if (si is not None and si.on_wait is not None
                        and len(si.on_wait) > 1
                        and tn != "InstEventSemaphore"
                        and eng is not None
                        and eng != mybir.EngineType.Unassigned):
                    waits = list(si.on_wait)
                    for w in waits[:-1]:
                        ev = mybir.InstEventSemaphore(
                            name=f"WSPLIT-{nid[0]}", ins=[], outs=[])
                        nid[0] += 1
                        ev.engine = eng
                        ev.sync_info = mybir.SyncInfo(on_wait=[w], on_update=[])
                        out.append(ev)
                    i.sync_info = mybir.SyncInfo(
                        on_wait=[waits[-1]], on_update=list(si.on_update))
                    changed = True
                out.append(i)
            if changed:
                blk.instructions = out


def _build(a_scales):
    """Emit the per-core program. a_scales[l][s] = A[l, d, s] (d-independent)."""
    nc = bass.Bass("TRN2", target_bir_lowering=False)

    # ---- dram inputs ----
    def din(name, shape, dt=F32):
        return nc.dram_tensor(name, shape, dt, kind="ExternalInput")

    xpatch = din("xpatch", [PATCH, NT])                  # per-core unfolded patches
    posW = din("posW", [PATCH, D_MODEL])
    posb = din("posb", [128, 2])                          # col = dm half
    posembT = din("posembT", [128, 2 * NPATCH])           # col = half*64+t
    rmsw = din("rmsw", [128, N_LAYERS * 2])               # col = l*2+half
    inW = din("inW", [128, N_LAYERS * 2 * 2 * D_INNER], BF16)   # (l,kb) major
    convw = din("convw", [128, N_LAYERS * 16])            # col = l*16+db*4+k
    convb = din("convb", [128, N_LAYERS * 4])             # col = l*4+db
    xpW = din("xpW", [128, N_LAYERS * 4 * XPD], BF16)     # (l,kb) major
    dtW = din("dtW", [DT_RANK, N_LAYERS * D_INNER], BF16)  # col = l*512+j
    dtb = din("dtb", [128, N_LAYERS * 4])
    Dskip = din("Dskip", [128, N_LAYERS * 4])
    outW = din("outW", [128, N_LAYERS * 4 * D_MODEL], BF16)  # (l,kb) major
    lng = din("lng", [128, 2])
    lnb = din("lnb", [128, 2])
    headW = din("headW", [128, KHEAD * PRED], BF16)       # col = kb*96+j
    headb = din("headb", [NSEQ, PRED])                    # host-replicated rows
    ones_in = din("ones", [128, 128])
    epsc = din("epsc", [128, 1])

    yout = nc.dram_tensor("yout", [NSEQ, PRED], F32, kind="ExternalOutput")

    with tile.TileContext(nc) as tc:
        import contextlib

        ctx = contextlib.ExitStack()
        with ctx:
            cp = ctx.enter_context(tc.tile_pool(name="consts", bufs=1))
            wp = ctx.enter_context(tc.tile_pool(name="work", bufs=1))
            ep = ctx.enter_context(tc.tile_pool(name="escan", bufs=2))
            pp = ctx.enter_context(tc.tile_pool(name="psum", bufs=3, space="PSUM"))
            pps = ctx.enter_context(tc.tile_pool(name="psum_s", bufs=2, space="PSUM"))
            dp = ctx.enter_context(tc.tile_pool(name="dram", bufs=2, space="DRAM"))

            # ---- load consts ----
            def cload(name, src, shape, dt=F32):
                t = cp.tile(shape, dt, tag=name, name=name)
                nc.sync.dma_start(t[:], src[:])
                return t

            posW_t = cload("posW", posW, [PATCH, D_MODEL])
            posb_t = cload("posb", posb, [128, 2])
            pose_t = cload("posembT", posembT, [128, 2 * NPATCH])
            rmsw_t = cload("rmsw", rmsw, [128, N_LAYERS * 2])
            inW_t = cload("inW", inW, [128, N_LAYERS * 2 * 2 * D_INNER], BF16)
            convw_t = cload("convw", convw, [128, N_LAYERS * 16])
            convb_t = cload("convb", convb, [128, N_LAYERS * 4])
            xpW_t = cload("xpW", xpW, [128, N_LAYERS * 4 * XPD], BF16)
            dtW_t = cload("dtW", dtW, [DT_RANK, N_LAYERS * D_INNER], BF16)
            dtb_t = cload("dtb", dtb, [128, N_LAYERS * 4])
            Dsk_t = cload("Dskip", Dskip, [128, N_LAYERS * 4])
            outW_t = cload("outW", outW, [128, N_LAYERS * 4 * D_MODEL], BF16)
            lng_t = cload("lng", lng, [128, 2])
            lnb_t = cload("lnb", lnb, [128, 2])
            headb_t = cload("headb", headb, [NSEQ, PRED])
            ones_t = cload("ones", ones_in, [128, 128])
            eps_t = cload("epsc", epsc, [128, 1])

            # patches rhs [16 partitions, (n,t)=896], unfolded host-side
            patches = cp.tile([PATCH, NT], F32, tag="patches", name="patches")
            nc.sync.dma_start(patches[:], xpatch[:])

            def nsl(nh):
                return slice(nh * NTH, (nh + 1) * NTH)

            def bcast_mid(ap2d, count):
                """[P, T] AP -> [P, count, T] with the middle dim broadcast."""
                aps = list(ap2d.ap)
                return bass.AP(ap2d.tensor, ap2d.offset,
                               [list(aps[0]), [0, count], list(aps[1])])

            # ---- positional encoding: h = patches @ posW + posb + posembT ----
            h = [wp.tile([128, NT], F32, tag=f"h{b}", name=f"h{b}") for b in range(2)]
            for b in range(2):
                for nh in range(NH):
                    ps = pp.tile([128, NTH], F32, tag="mm", name="mm")
                    nc.tensor.matmul(
                        ps[:], posW_t[:, b * 128:(b + 1) * 128],
                        patches[:, nsl(nh)], start=True, stop=True,
                    )
                    # h = psum + posb (per-partition) + posemb (bcast over n)
                    pe = bcast_mid(pose_t[:, b * NPATCH:(b + 1) * NPATCH], NSEQ // NH)
                    nc.vector.scalar_tensor_tensor(
                        h[b][:, nsl(nh)].rearrange("p (n t) -> p n t", t=NPATCH),
                        ps[:].rearrange("p (n t) -> p n t", t=NPATCH),
                        posb_t[:, b:b + 1],
                        pe,
                        AL.add, AL.add,
                    )

            def colnorm_rsqrt(rhs_tiles, scale, tag, want_sum=False):
                """Column variance-ish: rs_rep[p,c] = 1/sqrt(scale*sum_p(rhs) + EPS).

                rhs_tiles: two [128, NT] f32 tiles whose partition-sums to take
                (the ones-matmul replicates the sum to every partition).
                Returns ([128,NT] sum_rep f32 tile or None, [128,NT] rs_rep).
                """
                sum_rep = (wp.tile([128, NT], F32, tag="sumrep", name="sumrep")
                           if want_sum else None)
                rs_rep = wp.tile([128, NT], F32, tag="rsrep", name="rsrep")
                for nh in range(NH):
                    ps = pps.tile([128, NTH], F32, tag="red", name="red")
                    nc.tensor.matmul(ps[:], ones_t[:], rhs_tiles[0][:, nsl(nh)],
                                     start=True, stop=False)
                    nc.tensor.matmul(ps[:], ones_t[:], rhs_tiles[1][:, nsl(nh)],
                                     start=False, stop=True)
                    if want_sum:
                        nc.scalar.copy(sum_rep[:, nsl(nh)], ps[:])
                    # rsqrt via exp(-0.5*ln(x)); Rsqrt ACT table is blocked
                    nc.scalar.activation(rs_rep[:, nsl(nh)], ps[:], AF.Ln,
                                         bias=eps_t[:, 0:1], scale=scale)
                    nc.scalar.activation(rs_rep[:, nsl(nh)], rs_rep[:, nsl(nh)],
                                         AF.Exp, scale=-0.5)
                return sum_rep, rs_rep

            # =================== layers ===================
            for l in range(N_LAYERS):
                # ---- RMSNorm -> xn (bf16) ----
                hsq = [wp.tile([128, NT], F32, tag=f"hsq{b}", name=f"hsq{b}") for b in range(2)]
                for b in range(2):
                    nc.scalar.square(hsq[b][:], h[b][:])
                _, rs_rep = colnorm_rsqrt(hsq, 1.0 / D_MODEL, f"rms{l}")
                xn = [wp.tile([128, NT], BF16, tag=f"xn{b}", name=f"xn{b}") for b in range(2)]
                for b in range(2):
                    nc.vector.scalar_tensor_tensor(
                        xn[b][:], h[b][:], rmsw_t[:, l * 2 + b:l * 2 + b + 1],
                        rs_rep[:], AL.mult, AL.mult,
                    )

                # ---- in_proj -> v (pre-conv xi), sz (silu(z)) ----
                # u/sz/delta/up/yacc live as 4*NT merged tiles so the scan
                # phase can run one wide op over all four d-blocks.
                v = [wp.tile([128, NT], BF16, tag=f"v{db}", name=f"v{db}") for db in range(4)]
                sz_all = wp.tile([128, 4 * NT], BF16, tag="sz_all", name="sz_all")
                u_all = wp.tile([128, 4 * NT], BF16, tag="u_all", name="u_all")

                def dbsl(db, nh=None):
                    if nh is None:
                        return slice(db * NT, (db + 1) * NT)
                    return slice(db * NT + nh * NTH, db * NT + (nh + 1) * NTH)

                for mb in range(8):
                    for nh in range(NH):
                        ps = pp.tile([128, NTH], F32, tag="mm", name="mm")
                        for kb in range(2):
                            w0 = (l * 2 + kb) * (2 * D_INNER) + mb * 128
                            nc.tensor.matmul(
                                ps[:], inW_t[:, w0:w0 + 128], xn[kb][:, nsl(nh)],
                                start=(kb == 0), stop=(kb == 1),
                            )
                        if mb < 4:
                            nc.scalar.copy(v[mb][:, nsl(nh)], ps[:])
                        else:
                            nc.scalar.activation(sz_all[:, dbsl(mb - 4, nh)], ps[:],
                                                 AF.Silu)

                # ---- causal depthwise conv + silu -> u ----
                ca = [wp.tile([128, NT], BF16, tag=f"ca{db}", name=f"ca{db}") for db in range(4)]
                for db in range(4):
                    c0 = l * 16 + db * 4
                    nc.vector.tensor_scalar_mul(ca[db][:], v[db][:],
                                                convw_t[:, c0 + 3:c0 + 4])
                    cav = ca[db][:].rearrange("p (n t) -> p n t", t=NPATCH)
                    vv = v[db][:].rearrange("p (n t) -> p n t", t=NPATCH)
                    for k in range(1, D_CONV):
                        nc.vector.scalar_tensor_tensor(
                            cav[:, :, k:], vv[:, :, :NPATCH - k],
                            convw_t[:, c0 + 3 - k:c0 + 4 - k],
                            cav[:, :, k:], AL.mult, AL.add,
                        )
                    nc.scalar.activation(u_all[:, dbsl(db)], ca[db][:], AF.Silu,
                                         bias=convb_t[:, l * 4 + db:l * 4 + db + 1])

                # ---- x_proj -> bc = [dt; B; C] feature-major [48, NT] bf16 ----
                bc = wp.tile([XPD, NT], BF16, tag="bc", name="bc")
                for nh in range(NH):
                    ps = pp.tile([XPD, NTH], F32, tag="mm48", name="mm48", bufs=2)
                    for kb in range(4):
                        w0 = (l * 4 + kb) * XPD
                        nc.tensor.matmul(
                            ps[:], xpW_t[:, w0:w0 + XPD], u_all[:, dbsl(kb, nh)],
                            start=(kb == 0), stop=(kb == 3),
                        )
                    nc.scalar.copy(bc[:, nsl(nh)], ps[:])

                # ---- delta = softplus(dt @ dtW + dtb) f32 [512, NT] ----
                delta_all = wp.tile([128, 4 * NT], F32, tag="delta_all", name="delta_all")
                sptmp = wp.tile([128, NT], F32, tag="hsq0", name="sptmp")
                for db in range(4):
                    for nh in range(NH):
                        ps = pp.tile([128, NTH], F32, tag="mm", name="mm")
                        w0 = l * D_INNER + db * 128
                        nc.tensor.matmul(
                            ps[:], dtW_t[:, w0:w0 + 128], bc[0:DT_RANK, nsl(nh)],
                            start=True, stop=True,
                        )
                        # softplus(x) = ln(1 + exp(x)); Softplus has no ACT table
                        nc.scalar.activation(
                            sptmp[:, nsl(nh)], ps[:], AF.Exp,
                            bias=dtb_t[:, l * 4 + db:l * 4 + db + 1],
                        )
                        nc.scalar.activation(
                            delta_all[:, dbsl(db, nh)], sptmp[:, nsl(nh)], AF.Ln,
                            bias=1.0,
                        )

                # ---- u' = delta * u (bf16) ----
                up_all = wp.tile([128, 4 * NT], BF16, tag="up_all", name="up_all")
                nc.vector.tensor_mul(up_all[:], delta_all[:], u_all[:])

                # force dA=0 at sequence starts: delta[:, n*64] = large
                dv = delta_all[:].rearrange("p (n t) -> p n t", t=NPATCH)
                nc.vector.memset(dv[:, :, 0:1], 1.0e30)

                # B,C rows to DRAM for partition-replication
                bc_d = dp.tile([2 * D_STATE, NT], BF16, tag="bc_d", name="bc_d")
                nc.sync.dma_start(bc_d[:], bc[DT_RANK:XPD, :])

                # ---- selective scan over 16 states (one wide op per step) ----
                yacc_all = wp.tile([128, 4 * NT], BF16, tag="yacc_all", name="yacc_all")
                for s in range(D_STATE):
                    brep = ep.tile([128, 4 * NT], BF16, tag="brep", name="brep")
                    crep = ep.tile([128, 4 * NT], BF16, tag="crep", name="crep")
                    bsrc = bass.AP(bc_d[:].tensor, bc_d[:].offset + s * NT,
                                   [[0, 128], [0, 4], [1, NT]])
                    csrc = bass.AP(bc_d[:].tensor,
                                   bc_d[:].offset + (D_STATE + s) * NT,
                                   [[0, 128], [0, 4], [1, NT]])
                    nc.sync.dma_start(
                        brep[:].rearrange("p (q t) -> p q t", t=NT), bsrc)
                    nc.sync.dma_start(
                        crep[:].rearrange("p (q t) -> p q t", t=NT), csrc)
                    dA = ep.tile([128, 4 * NT], BF16, tag="dA", name="dA")
                    nc.scalar.activation(dA[:], delta_all[:], AF.Exp,
                                         scale=float(a_scales[l][s]))
                    dBx = ep.tile([128, 4 * NT], BF16, tag="dBx", name="dBx")
                    nc.vector.tensor_mul(dBx[:], up_all[:], brep[:])
                    hs = ep.tile([128, 4 * NT], BF16, tag="hs", name="hs")
                    nc.vector.tensor_tensor_scan(
                        hs[:], dA[:], dBx[:], 0.0, AL.mult, AL.add)
                    if s == 0:
                        nc.vector.tensor_mul(yacc_all[:], hs[:], crep[:])
                    else:
                        ch = ep.tile([128, 4 * NT], BF16, tag="dA", name="ch")
                        nc.vector.tensor_mul(ch[:], hs[:], crep[:])
                        nc.vector.tensor_add(yacc_all[:], yacc_all[:], ch[:])

                # ---- y = (u*Dskip + yacc) * sz ----
                yf_all = wp.tile([128, 4 * NT], BF16, tag="yf_all", name="yf_all")
                for db in range(4):
                    nc.vector.scalar_tensor_tensor(
                        yf_all[:, dbsl(db)], u_all[:, dbsl(db)],
                        Dsk_t[:, l * 4 + db:l * 4 + db + 1],
                        yacc_all[:, dbsl(db)], AL.mult, AL.add,
                    )
                nc.vector.tensor_mul(yf_all[:], yf_all[:], sz_all[:])

                # ---- out_proj + residual into h ----
                for mb in range(2):
                    for nh in range(NH):
                        ps = pp.tile([128, NTH], F32, tag="mm", name="mm")
                        for kb in range(4):
                            w0 = (l * 4 + kb) * D_MODEL + mb * 128
                            nc.tensor.matmul(
                                ps[:], outW_t[:, w0:w0 + 128], yf_all[:, dbsl(kb, nh)],
                                start=(kb == 0), stop=(kb == 3),
                            )
                        nc.vector.tensor_add(h[mb][:, nsl(nh)], h[mb][:, nsl(nh)], ps[:])

            # =================== final LayerNorm ===================
            hsq = [wp.tile([128, NT], F32, tag=f"hsq{b}", name=f"hsq{b}") for b in range(2)]
            for b in range(2):
                nc.scalar.square(hsq[b][:], h[b][:])
            msq_rep, _ = colnorm_rsqrt(hsq, 1.0 / D_MODEL, "lnsq", want_sum=True)
            mu_rep = wp.tile([128, NT], F32, tag="hsq1", name="murep")
            for nh in range(NH):
                ps = pps.tile([128, NTH], F32, tag="red", name="red")
                nc.tensor.matmul(ps[:], ones_t[:], h[0][:, nsl(nh)], start=True, stop=False)
                nc.tensor.matmul(ps[:], ones_t[:], h[1][:, nsl(nh)], start=False, stop=True)
                nc.scalar.mul(mu_rep[:, nsl(nh)], ps[:], 1.0 / D_MODEL)
            # var = msq/256 - mu^2; rs = rsqrt(var + eps)
            var = wp.tile([128, NT], F32, tag="hsq0", name="var")
            nc.vector.tensor_mul(var[:], mu_rep[:], mu_rep[:])
            nc.scalar.mul(msq_rep[:], msq_rep[:], 1.0 / D_MODEL)
            nc.vector.tensor_sub(var[:], msq_rep[:], var[:])
            rs_rep = wp.tile([128, NT], F32, tag="rsrep", name="lnrs")
            nc.scalar.activation(rs_rep[:], var[:], AF.Ln, bias=eps_t[:, 0:1])
            nc.scalar.activation(rs_rep[:], rs_rep[:], AF.Exp, scale=-0.5)

            hn = [wp.tile([128, NT], BF16, tag=f"hn{b}", name=f"hn{b}") for b in range(2)]
            hf = wp.tile([128, NT], F32, tag="sumrep", name="hf")
            for b in range(2):
                nc.vector.tensor_sub(hf[:], h[b][:], mu_rep[:])
                nc.vector.tensor_mul(hf[:], hf[:], rs_rep[:])
                nc.vector.tensor_scalar(
                    hn[b][:], hf[:], lng_t[:, b:b + 1], lnb_t[:, b:b + 1],
                    AL.mult, AL.add,
                )

            # =================== head (fp32) ===================
            psh_full = pps.tile([128, NTH], F32, tag="red", name="head")
            psh = psh_full[0:NSEQ, 0:PRED]
            HCH = 32  # kb-blocks per headW chunk
            for hc in range(KHEAD // HCH):
                headW_t = cp.tile([128, HCH * PRED], BF16, tag="headWc",
                                  name="headWc", bufs=2)
                nc.sync.dma_start(
                    headW_t[:], headW[:, hc * HCH * PRED:(hc + 1) * HCH * PRED])
                for j in range(HCH):
                    kb = hc * HCH + j
                    b = kb % 2
                    t = kb // 2
                    lhsT = bass.AP(
                        hn[b][:].tensor, hn[b][:].offset + t,
                        [[hn[b][:].ap[0][0], 128], [NPATCH, NSEQ]],
                    )
                    nc.tensor.matmul(
                        psh[:], lhsT, headW_t[:, j * PRED:(j + 1) * PRED],
                        start=(kb == 0), stop=(kb == KHEAD - 1),
                    )
            yo = wp.tile([NSEQ, PRED], F32, tag="yo", name="yo")
            nc.vector.tensor_add(yo[:], psh[:], headb_t[:])
            nc.sync.dma_start(yout[:], yo[:])

    _legalize_pe_waits(nc)
    return nc


def _prep_shared(inp):
    """Build the shared (replicated) input arrays from the full inputs."""
    f32 = np.float32
    bf = ml_dtypes.bfloat16
    out = {}
    out["posW"] = np.asarray(inp["pos_W"], f32)
    pb = np.zeros((128, 2), f32)
    pb[:, 0] = np.asarray(inp["pos_b"], f32)[:128]
    pb[:, 1] = np.asarray(inp["pos_b"], f32)[128:]
    out["posb"] = pb
    pe = np.asarray(inp["pos_emb"], f32)  # [64, 256]
    pet = np.zeros((128, 2 * NPATCH), f32)
    pet[:, :NPATCH] = pe[:, :128].T
    pet[:, NPATCH:] = pe[:, 128:].T
    out["posembT"] = pet
    rw = np.zeros((128, N_LAYERS * 2), f32)
    for l in range(N_LAYERS):
        rwl = np.asarray(inp["rms_w"], f32)[l]
        rw[:, l * 2] = rwl[:128]
        rw[:, l * 2 + 1] = rwl[128:]
    out["rmsw"] = rw
    iw = np.zeros((128, N_LAYERS * 2 * 2 * D_INNER), bf)
    for l in range(N_LAYERS):
        w = np.asarray(inp["in_proj_W"], f32)[l]  # [256, 1024]
        for kb in range(2):
            iw[:, (l * 2 + kb) * 2 * D_INNER:(l * 2 + kb + 1) * 2 * D_INNER] = \
                w[kb * 128:(kb + 1) * 128, :].astype(bf)
    out["inW"] = iw
    cw = np.zeros((128, N_LAYERS * 16), f32)
    cb = np.zeros((128, N_LAYERS * 4), f32)
    dtb_ = np.zeros((128, N_LAYERS * 4), f32)
    dsk = np.zeros((128, N_LAYERS * 4), f32)
    for l in range(N_LAYERS):
        cwl = np.asarray(inp["conv_W"], f32)[l][:, 0, :]  # [512, 4]
        cbl = np.asarray(inp["conv_b"], f32)[l]
        dbl = np.asarray(inp["dt_b"], f32)[l]
        dsl = np.asarray(inp["D_skip"], f32)[l]
        for db in range(4):
            cw[:, l * 16 + db * 4:l * 16 + db * 4 + 4] = cwl[db * 128:(db + 1) * 128, :]
            cb[:, l * 4 + db] = cbl[db * 128:(db + 1) * 128]
            dtb_[:, l * 4 + db] = dbl[db * 128:(db + 1) * 128]
            dsk[:, l * 4 + db] = dsl[db * 128:(db + 1) * 128]
    out["convw"] = cw
    out["convb"] = cb
    out["dtb"] = dtb_
    out["Dskip"] = dsk
    xw = np.zeros((128, N_LAYERS * 4 * XPD), bf)
    for l in range(N_LAYERS):
        w = np.asarray(inp["x_proj_W"], f32)[l]  # [512, 48]
        for kb in range(4):
            xw[:, (l * 4 + kb) * XPD:(l * 4 + kb + 1) * XPD] = \
                w[kb * 128:(kb + 1) * 128, :].astype(bf)
    out["xpW"] = xw
    dw = np.zeros((DT_RANK, N_LAYERS * D_INNER), bf)
    for l in range(N_LAYERS):
        dw[:, l * D_INNER:(l + 1) * D_INNER] = \
            np.asarray(inp["dt_W"], f32)[l].astype(bf)
    out["dtW"] = dw
    ow = np.zeros((128, N_LAYERS * 4 * D_MODEL), bf)
    for l in range(N_LAYERS):
        w = np.asarray(inp["out_proj_W"], f32)[l]  # [512, 256]
        for kb in range(4):
            ow[:, (l * 4 + kb) * D_MODEL:(l * 4 + kb + 1) * D_MODEL] = \
                w[kb * 128:(kb + 1) * 128, :].astype(bf)
    out["outW"] = ow
    lg = np.zeros((128, 2), f32)
    lb = np.zeros((128, 2), f32)
    lg[:, 0] = np.asarray(inp["ln_g"], f32)[:128]
    lg[:, 1] = np.asarray(inp["ln_g"], f32)[128:]
    lb[:, 0] = np.asarray(inp["ln_b"], f32)[:128]
    lb[:, 1] = np.asarray(inp["ln_b"], f32)[128:]
    out["lng"] = lg
    out["lnb"] = lb
    hw = np.asarray(inp["head_W"], f32)  # [16384, 96]
    out["headW"] = np.ascontiguousarray(
        hw.reshape(KHEAD, 128, PRED).transpose(1, 0, 2).reshape(
            128, KHEAD * PRED)).astype(bf)
    out["headb"] = np.broadcast_to(
        np.asarray(inp["head_b"], f32), (NSEQ, PRED)).copy()
    out["ones"] = np.ones((128, 128), f32)
    out["epsc"] = np.full((128, 1), EPS, f32)
    return out


def kernel(**inputs):
    x = np.asarray(inputs["x"], np.float32)          # [16, 7, 512]
    A = -np.exp(np.asarray(inputs["A_log"], np.float64))  # [2, 512, 16]
    # A is d-independent by construction; bake per-(l,s) scales as immediates
    a_scales = tuple(tuple(float(A[l, 0, s]) for s in range(D_STATE))
                     for l in range(N_LAYERS))

    key = a_scales
    if key not in _CACHE:
        _CACHE[key] = _build(a_scales)
    nc = _CACHE[key]

    shared = _prep_shared(inputs)
    xf = x.reshape(B * M, SEQ)
    xpad = np.concatenate([xf, np.repeat(xf[:, -1:], STRIDE, axis=1)], axis=1)
    idx = np.arange(NPATCH)[:, None] * STRIDE + np.arange(PATCH)[None, :]
    allpatch = xpad[:, idx]  # [112, 64, 16]

    in_maps = []
    for c in range(NCORES):
        m = dict(shared)
        pc = allpatch[c * NSEQ:(c + 1) * NSEQ]          # [14, 64, 16]
        m["xpatch"] = np.ascontiguousarray(
            pc.reshape(NT, PATCH).T, np.float32)         # [16, 896]
        in_maps.append(m)

    res = bass_utils.run_bass_kernel_spmd(nc, in_maps, core_ids=list(range(NCORES)))
    global LAST_RESULT
    LAST_RESULT = res
    outs = [res.results[c]["yout"] for c in range(NCORES)]
    y = np.concatenate(outs, axis=0)  # [112, 96]
    return y.reshape(B, M, PRED)


if __name__ == "__main__":
    import reference

    inp = {k: np.asarray(v) for k, v in reference.setup_inputs().items()}
    got = kernel(**inp)
    want = np.asarray(reference.reference(**inp))
    err = np.abs(got - want).max() / (np.abs(want).max() + 1e-30)
    print("Relative error:", err)



# revision 26
# speedup vs baseline: 1.3083x; 1.0201x over previous
"""Trainium2 Bass kernel for the patch-Mamba time-series model.

Sharding: data-parallel over the B*M=112 flattened batch axis across 8 cores
(14 sequences per core). All weights replicated.

Per-core layout: feature-major activations [feature-partitions, (seq,token) free].
The selective scan runs on the Vector engine via tensor_tensor_scan with the
recurrence chained along the free dim (sequence boundaries reset by forcing
dA=0 at t=0 of each sequence). dA = exp(A_s * delta) is produced by the Scalar
engine (one exp pass per state index, exploiting that A is d-independent).
"""

import sys

sys.path.insert(0, "/opt/trn_rl_repo")

import numpy as np
import ml_dtypes

import concourse.bass as bass
import concourse.mybir as mybir
import concourse.tile as tile
from concourse import bass_utils

F32 = mybir.dt.float32
BF16 = mybir.dt.bfloat16
AL = mybir.AluOpType
AF = mybir.ActivationFunctionType

# dims
B, M, SEQ = 16, 7, 512
PATCH, STRIDE, NPATCH = 16, 8, 64
D_MODEL, N_LAYERS, PRED = 256, 2, 96
D_INNER, D_STATE, DT_RANK, D_CONV = 512, 16, 16, 4
EPS = 1e-5
NCORES = 8
NSEQ = (B * M) // NCORES          # 14 sequences per core
NT = NSEQ * NPATCH                # 896 tokens per core
NH = 2                            # n-halves for matmul N<=512
NTH = NT // NH                    # 448
XPD = DT_RANK + 2 * D_STATE       # 48
KHEAD = (NPATCH * D_MODEL) // 128  # 128 k-blocks for the head

_CACHE = {}


def _legalize_pe_waits(nc):
    """walrus codegen accepts only ONE sync-wait on a PE Matmult (S3_LW
    struct); hoist extra waits onto standalone EventSemaphore carriers
    inserted immediately before the offending instruction."""
    nid = [0]
    for f in nc.m.functions:
        for blk in f.blocks:
            out = []
            changed = False
            for i in blk.instructions:
                si = getattr(i, "sync_info", None)
                tn = type(i).__name__
                eng = getattr(i, "engine", None)
                if (si is not None and si.on_wait is not None
                        and len(si.on_wait) > 1
                        and tn != "InstEventSemaphore"
                        and eng is not None
                        and eng != mybir.EngineType.Unassigned):
                    waits = list(si.on_wait)
                    for w in waits[:-1]:
                        ev = mybir.InstEventSemaphore(
                            name=f"WSPLIT-{nid[0]}", ins=[], outs=[])
                        nid[0] += 1
                        ev.engine = eng
                        ev.sync_info = mybir.SyncInfo(on_wait=[w], on_update=[])
                        out.append(ev)
                    i.sync_info = mybir.SyncInfo(
                        on_wait=[waits[-1]], on_update=list(si.on_update))
                    changed = True
                out.append(i)
            if changed:
                blk.instructions = out


def _build(a_scales):
    """Emit the per-core program. a_scales[l][s] = A[l, d, s] (d-independent)."""
    nc = bass.Bass("TRN2", target_bir_lowering=False)

    # ---- dram inputs ----
    def din(name, shape, dt=F32):
        return nc.dram_tensor(name, shape, dt, kind="ExternalInput")

    xpatch = din("xpatch", [PATCH, NT])                  # per-core unfolded patches
    posW = din("posW", [PATCH, D_MODEL])
    posb = din("posb", [128, 2])                          # col = dm half
    posembT = din("posembT", [128, 2 * NPATCH])           # col = half*64+t
    rmsw = din("rmsw", [128, N_LAYERS * 2])               # col = l*2+half
    inW = din("inW", [128, N_LAYERS * 2 * 2 * D_INNER], BF16)   # (l,kb) major
    convw = din("convw", [128, N_LAYERS * 16])            # col = l*16+db*4+k
    convb = din("convb", [128, N_LAYERS * 4])             # col = l*4+db
    xpW = din("xpW", [128, N_LAYERS * 4 * XPD], BF16)     # (l,kb) major
    dtW = din("dtW", [DT_RANK, N_LAYERS * D_INNER], BF16)  # col = l*512+j
    dtb = din("dtb", [128, N_LAYERS * 4])
    Dskip = din("Dskip", [128, N_LAYERS * 4])
    outW = din("outW", [128, N_LAYERS * 4 * D_MODEL], BF16)  # (l,kb) major
    lng = din("lng", [128, 2])
    lnb = din("lnb", [128, 2])
    headW = din("headW", [128, KHEAD * PRED], BF16)       # col = kb*96+j
    headb = din("headb", [NSEQ, PRED])                    # host-replicated rows
    ones_in = din("ones", [128, 128])
    epsc = din("epsc", [128, 1])

    yout = nc.dram_tensor("yout", [NSEQ, PRED], F32, kind="ExternalOutput")

    with tile.TileContext(nc) as tc:
        import contextlib

        ctx = contextlib.ExitStack()
        with ctx:
            cp = ctx.enter_context(tc.tile_pool(name="consts", bufs=1))
            wp = ctx.enter_context(tc.tile_pool(name="work", bufs=1))
            ep = ctx.enter_context(tc.tile_pool(name="escan", bufs=2))
            pp = ctx.enter_context(tc.tile_pool(name="psum", bufs=3, space="PSUM"))
            pps = ctx.enter_context(tc.tile_pool(name="psum_s", bufs=2, space="PSUM"))
            dp = ctx.enter_context(tc.tile_pool(name="dram", bufs=2, space="DRAM"))

            # ---- load consts ----
            def cload(name, src, shape, dt=F32):
                t = cp.tile(shape, dt, tag=name, name=name)
                nc.sync.dma_start(t[:], src[:])
                return t

            posW_t = cload("posW", posW, [PATCH, D_MODEL])
            posb_t = cload("posb", posb, [128, 2])
            pose_t = cload("posembT", posembT, [128, 2 * NPATCH])
            rmsw_t = cload("rmsw", rmsw, [128, N_LAYERS * 2])
            inW_t = cload("inW", inW, [128, N_LAYERS * 2 * 2 * D_INNER], BF16)
            convw_t = cload("convw", convw, [128, N_LAYERS * 16])
            convb_t = cload("convb", convb, [128, N_LAYERS * 4])
            xpW_t = cload("xpW", xpW, [128, N_LAYERS * 4 * XPD], BF16)
            dtW_t = cload("dtW", dtW, [DT_RANK, N_LAYERS * D_INNER], BF16)
            dtb_t = cload("dtb", dtb, [128, N_LAYERS * 4])
            Dsk_t = cload("Dskip", Dskip, [128, N_LAYERS * 4])
            outW_t = cload("outW", outW, [128, N_LAYERS * 4 * D_MODEL], BF16)
            lng_t = cload("lng", lng, [128, 2])
            lnb_t = cload("lnb", lnb, [128, 2])
            headb_t = cload("headb", headb, [NSEQ, PRED])
            ones_t = cload("ones", ones_in, [128, 128])
            eps_t = cload("epsc", epsc, [128, 1])

            # patches rhs [16 partitions, (n,t)=896], unfolded host-side
            patches = cp.tile([PATCH, NT], F32, tag="patches", name="patches")
            nc.sync.dma_start(patches[:], xpatch[:])

            def nsl(nh):
                return slice(nh * NTH, (nh + 1) * NTH)

            def bcast_mid(ap2d, count):
                """[P, T] AP -> [P, count, T] with the middle dim broadcast."""
                aps = list(ap2d.ap)
                return bass.AP(ap2d.tensor, ap2d.offset,
                               [list(aps[0]), [0, count], list(aps[1])])

            # ---- positional encoding: h = patches @ posW + posb + posembT ----
            h = [wp.tile([128, NT], F32, tag=f"h{b}", name=f"h{b}") for b in range(2)]
            for b in range(2):
                for nh in range(NH):
                    ps = pp.tile([128, NTH], F32, tag="mm", name="mm")
                    nc.tensor.matmul(
                        ps[:], posW_t[:, b * 128:(b + 1) * 128],
                        patches[:, nsl(nh)], start=True, stop=True,
                    )
                    # h = psum + posb (per-partition) + posemb (bcast over n)
                    pe = bcast_mid(pose_t[:, b * NPATCH:(b + 1) * NPATCH], NSEQ // NH)
                    nc.vector.scalar_tensor_tensor(
                        h[b][:, nsl(nh)].rearrange("p (n t) -> p n t", t=NPATCH),
                        ps[:].rearrange("p (n t) -> p n t", t=NPATCH),
                        posb_t[:, b:b + 1],
                        pe,
                        AL.add, AL.add,
                    )

            def colnorm_rsqrt(rhs_tiles, scale, tag, want_sum=False):
                """Column variance-ish: rs_rep[p,c] = 1/sqrt(scale*sum_p(rhs) + EPS).

                rhs_tiles: two [128, NT] f32 tiles whose partition-sums to take
                (the ones-matmul replicates the sum to every partition).
                Returns ([128,NT] sum_rep f32 tile or None, [128,NT] rs_rep).
                """
                sum_rep = (wp.tile([128, NT], F32, tag="sumrep", name="sumrep")
                           if want_sum else None)
                rs_rep = wp.tile([128, NT], F32, tag="rsrep", name="rsrep")
                for nh in range(NH):
                    ps = pps.tile([128, NTH], F32, tag="red", name="red")
                    nc.tensor.matmul(ps[:], ones_t[:], rhs_tiles[0][:, nsl(nh)],
                                     start=True, stop=False)
                    nc.tensor.matmul(ps[:], ones_t[:], rhs_tiles[1][:, nsl(nh)],
                                     start=False, stop=True)
                    if want_sum:
                        nc.scalar.copy(sum_rep[:, nsl(nh)], ps[:])
                    # rsqrt via exp(-0.5*ln(x)); Rsqrt ACT table is blocked
                    nc.scalar.activation(rs_rep[:, nsl(nh)], ps[:], AF.Ln,
                                         bias=eps_t[:, 0:1], scale=scale)
                    nc.scalar.activation(rs_rep[:, nsl(nh)], rs_rep[:, nsl(nh)],
                                         AF.Exp, scale=-0.5)
                return sum_rep, rs_rep

            # =================== layers ===================
            for l in range(N_LAYERS):
                # ---- RMSNorm -> xn (bf16) ----
                hsq = [wp.tile([128, NT], F32, tag=f"hsq{b}", name=f"hsq{b}") for b in range(2)]
                for b in range(2):
                    nc.scalar.square(hsq[b][:], h[b][:])
                _, rs_rep = colnorm_rsqrt(hsq, 1.0 / D_MODEL, f"rms{l}")
                xn = [wp.tile([128, NT], BF16, tag=f"xn{b}", name=f"xn{b}") for b in range(2)]
                for b in range(2):
                    nc.vector.scalar_tensor_tensor(
                        xn[b][:], h[b][:], rmsw_t[:, l * 2 + b:l * 2 + b + 1],
                        rs_rep[:], AL.mult, AL.mult,
                    )

                # ---- in_proj -> v (pre-conv xi), sz (silu(z)) ----
                # u/sz/delta/up/yacc live as 4*NT merged tiles so the scan
                # phase can run one wide op over all four d-blocks.
                v = [wp.tile([128, NT], BF16, tag=f"v{db}", name=f"v{db}") for db in range(4)]
                sz_all = wp.tile([128, 4 * NT], BF16, tag="sz_all", name="sz_all")
                u_all = wp.tile([128, 4 * NT], BF16, tag="u_all", name="u_all")

                def dbsl(db, nh=None):
                    if nh is None:
                        return slice(db * NT, (db + 1) * NT)
                    return slice(db * NT + nh * NTH, db * NT + (nh + 1) * NTH)

                for mb in range(8):
                    for nh in range(NH):
                        ps = pp.tile([128, NTH], F32, tag="mm", name="mm")
                        for kb in range(2):
                            w0 = (l * 2 + kb) * (2 * D_INNER) + mb * 128
                            nc.tensor.matmul(
                                ps[:], inW_t[:, w0:w0 + 128], xn[kb][:, nsl(nh)],
                                start=(kb == 0), stop=(kb == 1),
                            )
                        if mb < 4:
                            nc.scalar.copy(v[mb][:, nsl(nh)], ps[:])
                        else:
                            nc.scalar.activation(sz_all[:, dbsl(mb - 4, nh)], ps[:],
                                                 AF.Silu)

                # ---- causal depthwise conv + silu -> u ----
                ca = [wp.tile([128, NT], BF16, tag=f"ca{db}", name=f"ca{db}") for db in range(4)]
                for db in range(4):
                    c0 = l * 16 + db * 4
                    nc.vector.tensor_scalar_mul(ca[db][:], v[db][:],
                                                convw_t[:, c0 + 3:c0 + 4])
                    cav = ca[db][:].rearrange("p (n t) -> p n t", t=NPATCH)
                    vv = v[db][:].rearrange("p (n t) -> p n t", t=NPATCH)
                    for k in range(1, D_CONV):
                        nc.vector.scalar_tensor_tensor(
                            cav[:, :, k:], vv[:, :, :NPATCH - k],
                            convw_t[:, c0 + 3 - k:c0 + 4 - k],
                            cav[:, :, k:], AL.mult, AL.add,
                        )
                    nc.scalar.activation(u_all[:, dbsl(db)], ca[db][:], AF.Silu,
                                         bias=convb_t[:, l * 4 + db:l * 4 + db + 1])

                # ---- x_proj -> bc = [dt; B; C] feature-major [48, NT] bf16 ----
                bc = wp.tile([XPD, NT], BF16, tag="bc", name="bc")
                for nh in range(NH):
                    ps = pp.tile([XPD, NTH], F32, tag="mm48", name="mm48", bufs=2)
                    for kb in range(4):
                        w0 = (l * 4 + kb) * XPD
                        nc.tensor.matmul(
                            ps[:], xpW_t[:, w0:w0 + XPD], u_all[:, dbsl(kb, nh)],
                            start=(kb == 0), stop=(kb == 3),
                        )
                    nc.scalar.copy(bc[:, nsl(nh)], ps[:])

                # ---- delta = softplus(dt @ dtW + dtb) bf16 [512, NT] ----
                delta_all = wp.tile([128, 4 * NT], BF16, tag="delta_all", name="delta_all")
                sptmp = wp.tile([128, NT], BF16, tag="sptmp", name="sptmp")
                for db in range(4):
                    for nh in range(NH):
                        ps = pp.tile([128, NTH], F32, tag="mm", name="mm")
                        w0 = l * D_INNER + db * 128
                        nc.tensor.matmul(
                            ps[:], dtW_t[:, w0:w0 + 128], bc[0:DT_RANK, nsl(nh)],
                            start=True, stop=True,
                        )
                        # softplus(x) = ln(1 + exp(x)); Softplus has no ACT table
                        nc.scalar.activation(
                            sptmp[:, nsl(nh)], ps[:], AF.Exp,
                            bias=dtb_t[:, l * 4 + db:l * 4 + db + 1],
                        )
                        nc.scalar.activation(
                            delta_all[:, dbsl(db, nh)], sptmp[:, nsl(nh)], AF.Ln,
                            bias=1.0,
                        )

                # ---- u' = delta * u (bf16), per-db so each starts early ----
                up_all = wp.tile([128, 4 * NT], BF16, tag="up_all", name="up_all")
                for db in range(4):
                    nc.vector.tensor_mul(up_all[:, dbsl(db)],
                                         delta_all[:, dbsl(db)], u_all[:, dbsl(db)])

                # force dA=0 at sequence starts: delta[:, n*64] = large
                for db in range(4):
                    dv = delta_all[:, dbsl(db)].rearrange("p (n t) -> p n t", t=NPATCH)
                    nc.vector.memset(dv[:, :, 0:1], 1.0e30)

                # B,C rows to DRAM for partition-replication
                bc_d = dp.tile([2 * D_STATE, NT], BF16, tag="bc_d", name="bc_d")
                nc.sync.dma_start(bc_d[:], bc[DT_RANK:XPD, :])

                # ---- selective scan over 16 states (one wide op per step) ----
                yacc_all = wp.tile([128, 4 * NT], BF16, tag="yacc_all", name="yacc_all")
                for s in range(D_STATE):
                    brep = ep.tile([128, 4 * NT], BF16, tag="brep", name="brep")
                    crep = ep.tile([128, 4 * NT], BF16, tag="crep", name="crep")
                    bsrc = bass.AP(bc_d[:].tensor, bc_d[:].offset + s * NT,
                                   [[0, 128], [0, 4], [1, NT]])
                    csrc = bass.AP(bc_d[:].tensor,
                                   bc_d[:].offset + (D_STATE + s) * NT,
                                   [[0, 128], [0, 4], [1, NT]])
                    nc.sync.dma_start(
                        brep[:].rearrange("p (q t) -> p q t", t=NT), bsrc)
                    nc.sync.dma_start(
                        crep[:].rearrange("p (q t) -> p q t", t=NT), csrc)
                    dA = ep.tile([128, 4 * NT], BF16, tag="dA", name="dA")
                    dBx = ep.tile([128, 4 * NT], BF16, tag="dBx", name="dBx")
                    hs = ep.tile([128, 4 * NT], BF16, tag="hs", name="hs", bufs=1)
                    if s == 0:
                        # per-db so work starts before the last delta lands
                        for db in range(4):
                            nc.scalar.activation(dA[:, dbsl(db)],
                                                 delta_all[:, dbsl(db)], AF.Exp,
                                                 scale=float(a_scales[l][s]))
                            nc.vector.tensor_mul(dBx[:, dbsl(db)],
                                                 up_all[:, dbsl(db)],
                                                 brep[:, dbsl(db)])
                            nc.vector.tensor_tensor_scan(
                                hs[:, dbsl(db)], dA[:, dbsl(db)],
                                dBx[:, dbsl(db)], 0.0, AL.mult, AL.add)
                            nc.vector.tensor_mul(yacc_all[:, dbsl(db)],
                                                 hs[:, dbsl(db)], crep[:, dbsl(db)])
                    else:
                        nc.scalar.activation(dA[:], delta_all[:], AF.Exp,
                                             scale=float(a_scales[l][s]))
                        nc.vector.tensor_mul(dBx[:], up_all[:], brep[:])
                        nc.vector.tensor_tensor_scan(
                            hs[:], dA[:], dBx[:], 0.0, AL.mult, AL.add)
                        ch = ep.tile([128, 4 * NT], BF16, tag="ch", name="ch")
                        nc.vector.tensor_mul(ch[:], hs[:], crep[:])
                        nc.vector.tensor_add(yacc_all[:], yacc_all[:], ch[:])

                # ---- y = (u*Dskip + yacc) * sz ----
                yf_all = wp.tile([128, 4 * NT], BF16, tag="yf_all", name="yf_all")
                for db in range(4):
                    nc.vector.scalar_tensor_tensor(
                        yf_all[:, dbsl(db)], u_all[:, dbsl(db)],
                        Dsk_t[:, l * 4 + db:l * 4 + db + 1],
                        yacc_all[:, dbsl(db)], AL.mult, AL.add,
                    )
                nc.vector.tensor_mul(yf_all[:], yf_all[:], sz_all[:])

                # ---- out_proj + residual into h ----
                for mb in range(2):
                    for nh in range(NH):
                        ps = pp.tile([128, NTH], F32, tag="mm", name="mm")
                        for kb in range(4):
                            w0 = (l * 4 + kb) * D_MODEL + mb * 128
                            nc.tensor.matmul(
                                ps[:], outW_t[:, w0:w0 + 128], yf_all[:, dbsl(kb, nh)],
                                start=(kb == 0), stop=(kb == 3),
                            )
                        nc.vector.tensor_add(h[mb][:, nsl(nh)], h[mb][:, nsl(nh)], ps[:])

            # =================== final LayerNorm ===================
            hsq = [wp.tile([128, NT], F32, tag=f"hsq{b}", name=f"hsq{b}") for b in range(2)]
            for b in range(2):
                nc.scalar.square(hsq[b][:], h[b][:])
            msq_rep, _ = colnorm_rsqrt(hsq, 1.0 / D_MODEL, "lnsq", want_sum=True)
            mu_rep = wp.tile([128, NT], F32, tag="hsq1", name="murep")
            for nh in range(NH):
                ps = pps.tile([128, NTH], F32, tag="red", name="red")
                nc.tensor.matmul(ps[:], ones_t[:], h[0][:, nsl(nh)], start=True, stop=False)
                nc.tensor.matmul(ps[:], ones_t[:], h[1][:, nsl(nh)], start=False, stop=True)
                nc.scalar.mul(mu_rep[:, nsl(nh)], ps[:], 1.0 / D_MODEL)
            # var = msq/256 - mu^2; rs = rsqrt(var + eps)
            var = wp.tile([128, NT], F32, tag="hsq0", name="var")
            nc.vector.tensor_mul(var[:], mu_rep[:], mu_rep[:])
            nc.scalar.mul(msq_rep[:], msq_rep[:], 1.0 / D_MODEL)
            nc.vector.tensor_sub(var[:], msq_rep[:], var[:])
            rs_rep = wp.tile([128, NT], F32, tag="rsrep", name="lnrs")
            nc.scalar.activation(rs_rep[:], var[:], AF.Ln, bias=eps_t[:, 0:1])
            nc.scalar.activation(rs_rep[:], rs_rep[:], AF.Exp, scale=-0.5)

            hn = [wp.tile([128, NT], BF16, tag=f"hn{b}", name=f"hn{b}") for b in range(2)]
            hf = wp.tile([128, NT], F32, tag="sumrep", name="hf")
            for b in range(2):
                nc.vector.tensor_sub(hf[:], h[b][:], mu_rep[:])
                nc.vector.tensor_mul(hf[:], hf[:], rs_rep[:])
                nc.vector.tensor_scalar(
                    hn[b][:], hf[:], lng_t[:, b:b + 1], lnb_t[:, b:b + 1],
                    AL.mult, AL.add,
                )

            # =================== head (fp32) ===================
            psh_full = pps.tile([128, NTH], F32, tag="red", name="head")
            psh = psh_full[0:NSEQ, 0:PRED]
            HCH = 32  # kb-blocks per headW chunk
            for hc in range(KHEAD // HCH):
                headW_t = cp.tile([128, HCH * PRED], BF16, tag="headWc",
                                  name="headWc", bufs=2)
                nc.sync.dma_start(
                    headW_t[:], headW[:, hc * HCH * PRED:(hc + 1) * HCH * PRED])
                for j in sorted(range(HCH), key=lambda j: ((hc * HCH + j) % 2, j)):
                    kb = hc * HCH + j
                    b = kb % 2
                    t = kb // 2
                    lhsT = bass.AP(
                        hn[b][:].tensor, hn[b][:].offset + t,
                        [[hn[b][:].ap[0][0], 128], [NPATCH, NSEQ]],
                    )
                    nc.tensor.matmul(
                        psh[:], lhsT, headW_t[:, j * PRED:(j + 1) * PRED],
                        start=(kb == 0), stop=(kb == KHEAD - 1),
                    )
            yo = wp.tile([NSEQ, PRED], F32, tag="yo", name="yo")
            nc.vector.tensor_add(yo[:], psh[:], headb_t[:])
            nc.sync.dma_start(yout[:], yo[:])

    _legalize_pe_waits(nc)
    return nc


def _prep_shared(inp):
    """Build the shared (replicated) input arrays from the full inputs."""
    f32 = np.float32
    bf = ml_dtypes.bfloat16
    out = {}
    out["posW"] = np.asarray(inp["pos_W"], f32)
    pb = np.zeros((128, 2), f32)
    pb[:, 0] = np.asarray(inp["pos_b"], f32)[:128]
    pb[:, 1] = np.asarray(inp["pos_b"], f32)[128:]
    out["posb"] = pb
    pe = np.asarray(inp["pos_emb"], f32)  # [64, 256]
    pet = np.zeros((128, 2 * NPATCH), f32)
    pet[:, :NPATCH] = pe[:, :128].T
    pet[:, NPATCH:] = pe[:, 128:].T
    out["posembT"] = pet
    rw = np.zeros((128, N_LAYERS * 2), f32)
    for l in range(N_LAYERS):
        rwl = np.asarray(inp["rms_w"], f32)[l]
        rw[:, l * 2] = rwl[:128]
        rw[:, l * 2 + 1] = rwl[128:]
    out["rmsw"] = rw
    iw = np.zeros((128, N_LAYERS * 2 * 2 * D_INNER), bf)
    for l in range(N_LAYERS):
        w = np.asarray(inp["in_proj_W"], f32)[l]  # [256, 1024]
        for kb in range(2):
            iw[:, (l * 2 + kb) * 2 * D_INNER:(l * 2 + kb + 1) * 2 * D_INNER] = \
                w[kb * 128:(kb + 1) * 128, :].astype(bf)
    out["inW"] = iw
    cw = np.zeros((128, N_LAYERS * 16), f32)
    cb = np.zeros((128, N_LAYERS * 4), f32)
    dtb_ = np.zeros((128, N_LAYERS * 4), f32)
    dsk = np.zeros((128, N_LAYERS * 4), f32)
    for l in range(N_LAYERS):
        cwl = np.asarray(inp["conv_W"], f32)[l][:, 0, :]  # [512, 4]
        cbl = np.asarray(inp["conv_b"], f32)[l]
        dbl = np.asarray(inp["dt_b"], f32)[l]
        dsl = np.asarray(inp["D_skip"], f32)[l]
        for db in range(4):
            cw[:, l * 16 + db * 4:l * 16 + db * 4 + 4] = cwl[db * 128:(db + 1) * 128, :]
            cb[:, l * 4 + db] = cbl[db * 128:(db + 1) * 128]
            dtb_[:, l * 4 + db] = dbl[db * 128:(db + 1) * 128]
            dsk[:, l * 4 + db] = dsl[db * 128:(db + 1) * 128]
    out["convw"] = cw
    out["convb"] = cb
    out["dtb"] = dtb_
    out["Dskip"] = dsk
    xw = np.zeros((128, N_LAYERS * 4 * XPD), bf)
    for l in range(N_LAYERS):
        w = np.asarray(inp["x_proj_W"], f32)[l]  # [512, 48]
        for kb in range(4):
            xw[:, (l * 4 + kb) * XPD:(l * 4 + kb + 1) * XPD] = \
                w[kb * 128:(kb + 1) * 128, :].astype(bf)
    out["xpW"] = xw
    dw = np.zeros((DT_RANK, N_LAYERS * D_INNER), bf)
    for l in range(N_LAYERS):
        dw[:, l * D_INNER:(l + 1) * D_INNER] = \
            np.asarray(inp["dt_W"], f32)[l].astype(bf)
    out["dtW"] = dw
    ow = np.zeros((128, N_LAYERS * 4 * D_MODEL), bf)
    for l in range(N_LAYERS):
        w = np.asarray(inp["out_proj_W"], f32)[l]  # [512, 256]
        for kb in range(4):
            ow[:, (l * 4 + kb) * D_MODEL:(l * 4 + kb + 1) * D_MODEL] = \
                w[kb * 128:(kb + 1) * 128, :].astype(bf)
    out["outW"] = ow
    lg = np.zeros((128, 2), f32)
    lb = np.zeros((128, 2), f32)
    lg[:, 0] = np.asarray(inp["ln_g"], f32)[:128]
    lg[:, 1] = np.asarray(inp["ln_g"], f32)[128:]
    lb[:, 0] = np.asarray(inp["ln_b"], f32)[:128]
    lb[:, 1] = np.asarray(inp["ln_b"], f32)[128:]
    out["lng"] = lg
    out["lnb"] = lb
    hw = np.asarray(inp["head_W"], f32)  # [16384, 96]
    out["headW"] = np.ascontiguousarray(
        hw.reshape(KHEAD, 128, PRED).transpose(1, 0, 2).reshape(
            128, KHEAD * PRED)).astype(bf)
    out["headb"] = np.broadcast_to(
        np.asarray(inp["head_b"], f32), (NSEQ, PRED)).copy()
    out["ones"] = np.ones((128, 128), f32)
    out["epsc"] = np.full((128, 1), EPS, f32)
    return out


def kernel(**inputs):
    x = np.asarray(inputs["x"], np.float32)          # [16, 7, 512]
    A = -np.exp(np.asarray(inputs["A_log"], np.float64))  # [2, 512, 16]
    # A is d-independent by construction; bake per-(l,s) scales as immediates
    a_scales = tuple(tuple(float(A[l, 0, s]) for s in range(D_STATE))
                     for l in range(N_LAYERS))

    key = a_scales
    if key not in _CACHE:
        _CACHE[key] = _build(a_scales)
    nc = _CACHE[key]

    shared = _prep_shared(inputs)
    xf = x.reshape(B * M, SEQ)
    xpad = np.concatenate([xf, np.repeat(xf[:, -1:], STRIDE, axis=1)], axis=1)
    idx = np.arange(NPATCH)[:, None] * STRIDE + np.arange(PATCH)[None, :]
    allpatch = xpad[:, idx]  # [112, 64, 16]

    in_maps = []
    for c in range(NCORES):
        m = dict(shared)
        pc = allpatch[c * NSEQ:(c + 1) * NSEQ]          # [14, 64, 16]
        m["xpatch"] = np.ascontiguousarray(
            pc.reshape(NT, PATCH).T, np.float32)         # [16, 896]
        in_maps.append(m)

    res = bass_utils.run_bass_kernel_spmd(nc, in_maps, core_ids=list(range(NCORES)))
    global LAST_RESULT
    LAST_RESULT = res
    outs = [res.results[c]["yout"] for c in range(NCORES)]
    y = np.concatenate(outs, axis=0)  # [112, 96]
    return y.reshape(B, M, PRED)


if __name__ == "__main__":
    import reference

    inp = {k: np.asarray(v) for k, v in reference.setup_inputs().items()}
    got = kernel(**inp)
    want = np.asarray(reference.reference(**inp))
    err = np.abs(got - want).max() / (np.abs(want).max() + 1e-30)
    print("Relative error:", err)



# revision 37
# speedup vs baseline: 1.3169x; 1.0066x over previous
"""Trainium2 Bass kernel for the patch-Mamba time-series model.

Sharding: data-parallel over the B*M=112 flattened batch axis across 8 cores
(14 sequences per core). All weights replicated.

Per-core layout: feature-major activations [feature-partitions, (seq,token) free].
The selective scan runs on the Vector engine via tensor_tensor_scan with the
recurrence chained along the free dim (sequence boundaries reset by forcing
dA=0 at t=0 of each sequence). dA = exp(A_s * delta) is produced by the Scalar
engine (one exp pass per state index, exploiting that A is d-independent).
"""

import sys

sys.path.insert(0, "/opt/trn_rl_repo")

import numpy as np
import ml_dtypes

import concourse.bass as bass
import concourse.mybir as mybir
import concourse.tile as tile
from concourse import bass_utils

F32 = mybir.dt.float32
BF16 = mybir.dt.bfloat16
FP8 = mybir.dt.float8e4
AL = mybir.AluOpType
AF = mybir.ActivationFunctionType

# dims
B, M, SEQ = 16, 7, 512
PATCH, STRIDE, NPATCH = 16, 8, 64
D_MODEL, N_LAYERS, PRED = 256, 2, 96
D_INNER, D_STATE, DT_RANK, D_CONV = 512, 16, 16, 4
EPS = 1e-5
NCORES = 8
NSEQ = (B * M) // NCORES          # 14 sequences per core
NT = NSEQ * NPATCH                # 896 tokens per core
NH = 2                            # n-halves for matmul N<=512
NTH = NT // NH                    # 448
XPD = DT_RANK + 2 * D_STATE       # 48
KHEAD = (NPATCH * D_MODEL) // 128  # 128 k-blocks for the head

_CACHE = {}


def _legalize_pe_waits(nc):
    """walrus codegen accepts only ONE sync-wait on a PE Matmult (S3_LW
    struct); hoist extra waits onto standalone EventSemaphore carriers
    inserted immediately before the offending instruction."""
    nid = [0]
    for f in nc.m.functions:
        for blk in f.blocks:
            out = []
            changed = False
            for i in blk.instructions:
                si = getattr(i, "sync_info", None)
                tn = type(i).__name__
                eng = getattr(i, "engine", None)
                if (si is not None and si.on_wait is not None
                        and len(si.on_wait) > 1
                        and tn != "InstEventSemaphore"
                        and eng is not None
                        and eng != mybir.EngineType.Unassigned):
                    waits = list(si.on_wait)
                    for w in waits[:-1]:
                        ev = mybir.InstEventSemaphore(
                            name=f"WSPLIT-{nid[0]}", ins=[], outs=[])
                        nid[0] += 1
                        ev.engine = eng
                        ev.sync_info = mybir.SyncInfo(on_wait=[w], on_update=[])
                        out.append(ev)
                    i.sync_info = mybir.SyncInfo(
                        on_wait=[waits[-1]], on_update=list(si.on_update))
                    changed = True
                out.append(i)
            if changed:
                blk.instructions = out


def _build(a_scales):
    """Emit the per-core program. a_scales[l][s] = A[l, d, s] (d-independent)."""
    nc = bass.Bass("TRN2", target_bir_lowering=False)

    # ---- dram inputs ----
    def din(name, shape, dt=F32):
        return nc.dram_tensor(name, shape, dt, kind="ExternalInput")

    xpatch = din("xpatch", [PATCH, NT])                  # per-core unfolded patches
    posW = din("posW", [PATCH, D_MODEL])
    posb = din("posb", [128, 2])                          # col = dm half
    posembT = din("posembT", [128, 2 * NPATCH])           # col = half*64+t
    rmsw = din("rmsw", [128, N_LAYERS * 2])               # col = l*2+half
    inW = din("inW", [128, N_LAYERS * 2 * 2 * D_INNER], BF16)   # (l,kb) major
    convw = din("convw", [128, N_LAYERS * 16])            # col = l*16+db*4+k
    convb = din("convb", [128, N_LAYERS * 4])             # col = l*4+db
    xpW = din("xpW", [128, N_LAYERS * 4 * XPD], BF16)     # (l,kb) major
    dtW = din("dtW", [DT_RANK, N_LAYERS * D_INNER], BF16)  # col = l*512+j
    dtb = din("dtb", [128, N_LAYERS * 4])
    Dskip = din("Dskip", [128, N_LAYERS * 4])
    outW = din("outW", [128, N_LAYERS * 4 * D_MODEL], BF16)  # (l,kb) major
    lng = din("lng", [128, 2])
    lnb = din("lnb", [128, 2])
    headW = din("headW", [128, KHEAD * PRED], BF16)       # col = kb*96+j
    headb = din("headb", [NSEQ, PRED])                    # host-replicated rows
    ones_in = din("ones", [128, 128])
    epsc = din("epsc", [128, 1])

    yout = nc.dram_tensor("yout", [NSEQ, PRED], F32, kind="ExternalOutput")

    with tile.TileContext(nc) as tc:
        import contextlib

        ctx = contextlib.ExitStack()
        with ctx:
            cp = ctx.enter_context(tc.tile_pool(name="consts", bufs=1))
            wp = ctx.enter_context(tc.tile_pool(name="work", bufs=1))
            ep = ctx.enter_context(tc.tile_pool(name="escan", bufs=2))
            pp = ctx.enter_context(tc.tile_pool(name="psum", bufs=3, space="PSUM"))
            pps = ctx.enter_context(tc.tile_pool(name="psum_s", bufs=2, space="PSUM"))
            dp = ctx.enter_context(tc.tile_pool(name="dram", bufs=2, space="DRAM"))

            # ---- load consts ----
            def cload(name, src, shape, dt=F32):
                t = cp.tile(shape, dt, tag=name, name=name)
                nc.sync.dma_start(t[:], src[:])
                return t

            # patches + pos consts first so the pos phase isn't queued
            # behind the ~2MB of layer weights on the DMA queue
            patches = cp.tile([PATCH, NT], F32, tag="patches", name="patches")
            nc.sync.dma_start(patches[:], xpatch[:])
            posW_t = cload("posW", posW, [PATCH, D_MODEL])
            posb_t = cload("posb", posb, [128, 2])
            pose_t = cload("posembT", posembT, [128, 2 * NPATCH])
            rmsw_t = cload("rmsw", rmsw, [128, N_LAYERS * 2])
            inW_t = cload("inW", inW, [128, N_LAYERS * 2 * 2 * D_INNER], BF16)
            convw_t = cload("convw", convw, [128, N_LAYERS * 16])
            convb_t = cload("convb", convb, [128, N_LAYERS * 4])
            xpW_t = cload("xpW", xpW, [128, N_LAYERS * 4 * XPD], BF16)
            dtW_t = cload("dtW", dtW, [DT_RANK, N_LAYERS * D_INNER], BF16)
            dtb_t = cload("dtb", dtb, [128, N_LAYERS * 4])
            Dsk_t = cload("Dskip", Dskip, [128, N_LAYERS * 4])
            outW_t = cload("outW", outW, [128, N_LAYERS * 4 * D_MODEL], BF16)
            lng_t = cload("lng", lng, [128, 2])
            lnb_t = cload("lnb", lnb, [128, 2])
            headb_t = cload("headb", headb, [NSEQ, PRED])
            ones_t = cload("ones", ones_in, [128, 128])
            eps_t = cload("epsc", epsc, [128, 1])

            def nsl(nh):
                return slice(nh * NTH, (nh + 1) * NTH)

            def bcast_mid(ap2d, count):
                """[P, T] AP -> [P, count, T] with the middle dim broadcast."""
                aps = list(ap2d.ap)
                return bass.AP(ap2d.tensor, ap2d.offset,
                               [list(aps[0]), [0, count], list(aps[1])])

            # ---- positional encoding: h = patches @ posW + posb + posembT ----
            h = [wp.tile([128, NT], F32, tag=f"h{b}", name=f"h{b}") for b in range(2)]
            for b in range(2):
                for nh in range(NH):
                    ps = pp.tile([128, NTH], F32, tag="mm", name="mm")
                    nc.tensor.matmul(
                        ps[:], posW_t[:, b * 128:(b + 1) * 128],
                        patches[:, nsl(nh)], start=True, stop=True,
                    )
                    # h = psum + posb (per-partition) + posemb (bcast over n)
                    pe = bcast_mid(pose_t[:, b * NPATCH:(b + 1) * NPATCH], NSEQ // NH)
                    nc.vector.scalar_tensor_tensor(
                        h[b][:, nsl(nh)].rearrange("p (n t) -> p n t", t=NPATCH),
                        ps[:].rearrange("p (n t) -> p n t", t=NPATCH),
                        posb_t[:, b:b + 1],
                        pe,
                        AL.add, AL.add,
                    )

            def colnorm_rsqrt(rhs_tiles, scale, tag, want_sum=False):
                """Column variance-ish: rs_rep[p,c] = 1/sqrt(scale*sum_p(rhs) + EPS).

                rhs_tiles: two [128, NT] f32 tiles whose partition-sums to take
                (the ones-matmul replicates the sum to every partition).
                Returns ([128,NT] sum_rep f32 tile or None, [128,NT] rs_rep).
                """
                sum_rep = (wp.tile([128, NT], F32, tag="sumrep", name="sumrep")
                           if want_sum else None)
                rs_rep = wp.tile([128, NT], F32, tag="rsrep", name="rsrep")
                for nh in range(NH):
                    ps = pps.tile([128, NTH], F32, tag="red", name="red")
                    nc.tensor.matmul(ps[:], ones_t[:], rhs_tiles[0][:, nsl(nh)],
                                     start=True, stop=False)
                    nc.tensor.matmul(ps[:], ones_t[:], rhs_tiles[1][:, nsl(nh)],
                                     start=False, stop=True)
                    if want_sum:
                        nc.scalar.copy(sum_rep[:, nsl(nh)], ps[:])
                    # rsqrt via exp(-0.5*ln(x)); Rsqrt ACT table is blocked
                    nc.scalar.activation(rs_rep[:, nsl(nh)], ps[:], AF.Ln,
                                         bias=eps_t[:, 0:1], scale=scale)
                    nc.scalar.activation(rs_rep[:, nsl(nh)], rs_rep[:, nsl(nh)],
                                         AF.Exp, scale=-0.5)
                return sum_rep, rs_rep

            # =================== layers ===================
            for l in range(N_LAYERS):
                # ---- RMSNorm -> xn (bf16) ----
                hsq = [wp.tile([128, NT], F32, tag=f"hsq{b}", name=f"hsq{b}") for b in range(2)]
                for b in range(2):
                    nc.scalar.square(hsq[b][:], h[b][:])
                _, rs_rep = colnorm_rsqrt(hsq, 1.0 / D_MODEL, f"rms{l}")
                xn = [wp.tile([128, NT], BF16, tag=f"xn{b}", name=f"xn{b}") for b in range(2)]
                for b in range(2):
                    nc.vector.scalar_tensor_tensor(
                        xn[b][:], h[b][:], rmsw_t[:, l * 2 + b:l * 2 + b + 1],
                        rs_rep[:], AL.mult, AL.mult,
                    )

                # ---- in_proj -> v (pre-conv xi), sz (silu(z)) ----
                # u/sz/delta/up/yacc live as 4*NT merged tiles so the scan
                # phase can run one wide op over all four d-blocks.
                v = [wp.tile([128, NT], BF16, tag=f"v{db}", name=f"v{db}") for db in range(4)]
                sz_all = wp.tile([128, 4 * NT], BF16, tag="sz_all", name="sz_all")
                u_all = wp.tile([128, 4 * NT], BF16, tag="u_all", name="u_all")

                def dbsl(db, nh=None):
                    if nh is None:
                        return slice(db * NT, (db + 1) * NT)
                    return slice(db * NT + nh * NTH, db * NT + (nh + 1) * NTH)

                for mb in range(8):
                    for nh in range(NH):
                        ps = pp.tile([128, NTH], F32, tag="mm", name="mm")
                        for kb in range(2):
                            w0 = (l * 2 + kb) * (2 * D_INNER) + mb * 128
                            nc.tensor.matmul(
                                ps[:], inW_t[:, w0:w0 + 128], xn[kb][:, nsl(nh)],
                                start=(kb == 0), stop=(kb == 1),
                            )
                        if mb < 4:
                            nc.scalar.copy(v[mb][:, nsl(nh)], ps[:])
                        else:
                            nc.scalar.activation(sz_all[:, dbsl(mb - 4, nh)], ps[:],
                                                 AF.Silu)

                # ---- causal depthwise conv + silu -> u ----
                ca = [wp.tile([128, NT], BF16, tag=f"ca{db}", name=f"ca{db}") for db in range(4)]
                for db in range(4):
                    c0 = l * 16 + db * 4
                    nc.vector.tensor_scalar_mul(ca[db][:], v[db][:],
                                                convw_t[:, c0 + 3:c0 + 4])
                    cav = ca[db][:].rearrange("p (n t) -> p n t", t=NPATCH)
                    vv = v[db][:].rearrange("p (n t) -> p n t", t=NPATCH)
                    for k in range(1, D_CONV):
                        nc.vector.scalar_tensor_tensor(
                            cav[:, :, k:], vv[:, :, :NPATCH - k],
                            convw_t[:, c0 + 3 - k:c0 + 4 - k],
                            cav[:, :, k:], AL.mult, AL.add,
                        )
                    nc.scalar.activation(u_all[:, dbsl(db)], ca[db][:], AF.Silu,
                                         bias=convb_t[:, l * 4 + db:l * 4 + db + 1])

                # ---- x_proj -> bc = [dt; B; C] feature-major [48, NT] bf16 ----
                bc = wp.tile([XPD, NT], BF16, tag="bc", name="bc")
                for nh in range(NH):
                    ps = pp.tile([XPD, NTH], F32, tag="mm48", name="mm48", bufs=2)
                    for kb in range(4):
                        w0 = (l * 4 + kb) * XPD
                        nc.tensor.matmul(
                            ps[:], xpW_t[:, w0:w0 + XPD], u_all[:, dbsl(kb, nh)],
                            start=(kb == 0), stop=(kb == 3),
                        )
                    nc.scalar.copy(bc[:, nsl(nh)], ps[:])

                # ---- delta = softplus(dt @ dtW + dtb) bf16 [512, NT] ----
                delta_all = wp.tile([128, 4 * NT], BF16, tag="delta_all", name="delta_all")
                sptmp = wp.tile([128, NT], BF16, tag="sptmp", name="sptmp")
                for db in range(4):
                    for nh in range(NH):
                        ps = pp.tile([128, NTH], F32, tag="mm", name="mm")
                        w0 = l * D_INNER + db * 128
                        nc.tensor.matmul(
                            ps[:], dtW_t[:, w0:w0 + 128], bc[0:DT_RANK, nsl(nh)],
                            start=True, stop=True,
                        )
                        # softplus(x) = ln(1 + exp(x)); Softplus has no ACT table
                        nc.scalar.activation(
                            sptmp[:, nsl(nh)], ps[:], AF.Exp,
                            bias=dtb_t[:, l * 4 + db:l * 4 + db + 1],
                        )
                        nc.scalar.activation(
                            delta_all[:, dbsl(db, nh)], sptmp[:, nsl(nh)], AF.Ln,
                            bias=1.0,
                        )

                # ---- u' = delta * u (bf16), per-db so each starts early ----
                up_all = wp.tile([128, 4 * NT], BF16, tag="up_all", name="up_all")
                for db in range(4):
                    nc.vector.tensor_mul(up_all[:, dbsl(db)],
                                         delta_all[:, dbsl(db)], u_all[:, dbsl(db)])

                # force dA=0 at sequence starts: delta[:, n*64] = large
                for db in range(4):
                    dv = delta_all[:, dbsl(db)].rearrange("p (n t) -> p n t", t=NPATCH)
                    nc.vector.memset(dv[:, :, 0:1], 1.0e30)

                # B,C rows to DRAM for partition-replication
                bc_d = dp.tile([2 * D_STATE, NT], BF16, tag="bc_d", name="bc_d")
                nc.sync.dma_start(bc_d[:], bc[DT_RANK:XPD, :])

                # ---- selective scan over 16 states (one wide op per step) ----
                yacc_all = wp.tile([128, 4 * NT], BF16, tag="yacc_all", name="yacc_all")
                for s in range(D_STATE):
                    brep = ep.tile([128, 4 * NT], BF16, tag="brep", name="brep",
                                   bufs=3)
                    crep = ep.tile([128, 4 * NT], BF16, tag="crep", name="crep")
                    bsrc = bass.AP(bc_d[:].tensor, bc_d[:].offset + s * NT,
                                   [[0, 128], [0, 4], [1, NT]])
                    csrc = bass.AP(bc_d[:].tensor,
                                   bc_d[:].offset + (D_STATE + s) * NT,
                                   [[0, 128], [0, 4], [1, NT]])
                    nc.sync.dma_start(
                        brep[:].rearrange("p (q t) -> p q t", t=NT), bsrc)
                    nc.sync.dma_start(
                        crep[:].rearrange("p (q t) -> p q t", t=NT), csrc)
                    dA = ep.tile([128, 4 * NT], BF16, tag="dA", name="dA")
                    dBx = ep.tile([128, 4 * NT], BF16, tag="dBx", name="dBx", bufs=1)
                    hs = ep.tile([128, 4 * NT], BF16, tag="hs", name="hs", bufs=1)
                    if s == 0:
                        # per-db so work starts before the last delta lands
                        for db in range(4):
                            nc.scalar.activation(dA[:, dbsl(db)],
                                                 delta_all[:, dbsl(db)], AF.Exp,
                                                 scale=float(a_scales[l][s]))
                            nc.vector.tensor_mul(dBx[:, dbsl(db)],
                                                 up_all[:, dbsl(db)],
                                                 brep[:, dbsl(db)])
                            nc.vector.tensor_tensor_scan(
                                hs[:, dbsl(db)], dA[:, dbsl(db)],
                                dBx[:, dbsl(db)], 0.0, AL.mult, AL.add)
                            nc.vector.tensor_mul(yacc_all[:, dbsl(db)],
                                                 hs[:, dbsl(db)], crep[:, dbsl(db)])
                    else:
                        nc.scalar.activation(dA[:], delta_all[:], AF.Exp,
                                             scale=float(a_scales[l][s]))
                        nc.vector.tensor_mul(dBx[:], up_all[:], brep[:])
                        nc.vector.tensor_tensor_scan(
                            hs[:], dA[:], dBx[:], 0.0, AL.mult, AL.add)
                        ch = ep.tile([128, 4 * NT], BF16, tag="ch", name="ch")
                        nc.vector.tensor_mul(ch[:], hs[:], crep[:])
                        nc.vector.tensor_add(yacc_all[:], yacc_all[:], ch[:])

                # ---- y = (u*Dskip + yacc) * sz ----
                yf_all = wp.tile([128, 4 * NT], BF16, tag="yf_all", name="yf_all")
                for db in range(4):
                    nc.vector.scalar_tensor_tensor(
                        yf_all[:, dbsl(db)], u_all[:, dbsl(db)],
                        Dsk_t[:, l * 4 + db:l * 4 + db + 1],
                        yacc_all[:, dbsl(db)], AL.mult, AL.add,
                    )
                nc.vector.tensor_mul(yf_all[:], yf_all[:], sz_all[:])

                # ---- out_proj + residual into h ----
                for mb in range(2):
                    for nh in range(NH):
                        ps = pp.tile([128, NTH], F32, tag="mm", name="mm")
                        for kb in range(4):
                            w0 = (l * 4 + kb) * D_MODEL + mb * 128
                            nc.tensor.matmul(
                                ps[:], outW_t[:, w0:w0 + 128], yf_all[:, dbsl(kb, nh)],
                                start=(kb == 0), stop=(kb == 3),
                            )
                        nc.vector.tensor_add(h[mb][:, nsl(nh)], h[mb][:, nsl(nh)], ps[:])

            # =================== final LayerNorm ===================
            hsq = [wp.tile([128, NT], F32, tag=f"hsq{b}", name=f"hsq{b}") for b in range(2)]
            for b in range(2):
                nc.scalar.square(hsq[b][:], h[b][:])
            msq_rep, _ = colnorm_rsqrt(hsq, 1.0 / D_MODEL, "lnsq", want_sum=True)
            mu_rep = wp.tile([128, NT], F32, tag="hsq1", name="murep")
            for nh in range(NH):
                ps = pps.tile([128, NTH], F32, tag="red", name="red")
                nc.tensor.matmul(ps[:], ones_t[:], h[0][:, nsl(nh)], start=True, stop=False)
                nc.tensor.matmul(ps[:], ones_t[:], h[1][:, nsl(nh)], start=False, stop=True)
                nc.scalar.mul(mu_rep[:, nsl(nh)], ps[:], 1.0 / D_MODEL)
            # var = msq/256 - mu^2; rs = rsqrt(var + eps)
            var = wp.tile([128, NT], F32, tag="hsq0", name="var")
            nc.vector.tensor_mul(var[:], mu_rep[:], mu_rep[:])
            nc.scalar.mul(msq_rep[:], msq_rep[:], 1.0 / D_MODEL)
            nc.vector.tensor_sub(var[:], msq_rep[:], var[:])
            rs_rep = wp.tile([128, NT], F32, tag="rsrep", name="lnrs")
            nc.scalar.activation(rs_rep[:], var[:], AF.Ln, bias=eps_t[:, 0:1])
            nc.scalar.activation(rs_rep[:], rs_rep[:], AF.Exp, scale=-0.5)

            hn = [wp.tile([128, NT], BF16, tag=f"hn{b}", name=f"hn{b}") for b in range(2)]
            hf = wp.tile([128, NT], F32, tag="sumrep", name="hf")
            for b in range(2):
                nc.vector.tensor_sub(hf[:], h[b][:], mu_rep[:])
                nc.vector.tensor_mul(hf[:], hf[:], rs_rep[:])
                nc.vector.tensor_scalar(
                    hn[b][:], hf[:], lng_t[:, b:b + 1], lnb_t[:, b:b + 1],
                    AL.mult, AL.add,
                )

            # =================== head (bf16) ===================
            psh_full = pps.tile([128, NTH], F32, tag="red", name="head")
            psh = psh_full[0:NSEQ, 0:PRED]
            HCH = 32  # kb-blocks per headW chunk
            for hc in range(KHEAD // HCH):
                headW_t = cp.tile([128, HCH * PRED], BF16, tag="headWc",
                                  name="headWc", bufs=2)
                nc.sync.dma_start(
                    headW_t[:], headW[:, hc * HCH * PRED:(hc + 1) * HCH * PRED])
                for j in sorted(range(HCH), key=lambda j: ((hc * HCH + j) % 2, j)):
                    kb = hc * HCH + j
                    b = kb % 2
                    t = kb // 2
                    lhsT = bass.AP(
                        hn[b][:].tensor, hn[b][:].offset + t,
                        [[hn[b][:].ap[0][0], 128], [NPATCH, NSEQ]],
                    )
                    nc.tensor.matmul(
                        psh[:], lhsT, headW_t[:, j * PRED:(j + 1) * PRED],
                        start=(kb == 0), stop=(kb == KHEAD - 1),
                    )
            yo = wp.tile([NSEQ, PRED], F32, tag="yo", name="yo")
            nc.vector.tensor_add(yo[:], psh[:], headb_t[:])
            nc.sync.dma_start(yout[:], yo[:])

    _legalize_pe_waits(nc)
    return nc


def _prep_shared(inp):
    """Build the shared (replicated) input arrays from the full inputs."""
    f32 = np.float32
    bf = ml_dtypes.bfloat16
    out = {}
    out["posW"] = np.asarray(inp["pos_W"], f32)
    pb = np.zeros((128, 2), f32)
    pb[:, 0] = np.asarray(inp["pos_b"], f32)[:128]
    pb[:, 1] = np.asarray(inp["pos_b"], f32)[128:]
    out["posb"] = pb
    pe = np.asarray(inp["pos_emb"], f32)  # [64, 256]
    pet = np.zeros((128, 2 * NPATCH), f32)
    pet[:, :NPATCH] = pe[:, :128].T
    pet[:, NPATCH:] = pe[:, 128:].T
    out["posembT"] = pet
    rw = np.zeros((128, N_LAYERS * 2), f32)
    for l in range(N_LAYERS):
        rwl = np.asarray(inp["rms_w"], f32)[l]
        rw[:, l * 2] = rwl[:128]
        rw[:, l * 2 + 1] = rwl[128:]
    out["rmsw"] = rw
    iw = np.zeros((128, N_LAYERS * 2 * 2 * D_INNER), bf)
    for l in range(N_LAYERS):
        w = np.asarray(inp["in_proj_W"], f32)[l]  # [256, 1024]
        for kb in range(2):
            iw[:, (l * 2 + kb) * 2 * D_INNER:(l * 2 + kb + 1) * 2 * D_INNER] = \
                w[kb * 128:(kb + 1) * 128, :].astype(bf)
    out["inW"] = iw
    cw = np.zeros((128, N_LAYERS * 16), f32)
    cb = np.zeros((128, N_LAYERS * 4), f32)
    dtb_ = np.zeros((128, N_LAYERS * 4), f32)
    dsk = np.zeros((128, N_LAYERS * 4), f32)
    for l in range(N_LAYERS):
        cwl = np.asarray(inp["conv_W"], f32)[l][:, 0, :]  # [512, 4]
        cbl = np.asarray(inp["conv_b"], f32)[l]
        dbl = np.asarray(inp["dt_b"], f32)[l]
        dsl = np.asarray(inp["D_skip"], f32)[l]
        for db in range(4):
            cw[:, l * 16 + db * 4:l * 16 + db * 4 + 4] = cwl[db * 128:(db + 1) * 128, :]
            cb[:, l * 4 + db] = cbl[db * 128:(db + 1) * 128]
            dtb_[:, l * 4 + db] = dbl[db * 128:(db + 1) * 128]
            dsk[:, l * 4 + db] = dsl[db * 128:(db + 1) * 128]
    out["convw"] = cw
    out["convb"] = cb
    out["dtb"] = dtb_
    out["Dskip"] = dsk
    xw = np.zeros((128, N_LAYERS * 4 * XPD), bf)
    for l in range(N_LAYERS):
        w = np.asarray(inp["x_proj_W"], f32)[l]  # [512, 48]
        for kb in range(4):
            xw[:, (l * 4 + kb) * XPD:(l * 4 + kb + 1) * XPD] = \
                w[kb * 128:(kb + 1) * 128, :].astype(bf)
    out["xpW"] = xw
    dw = np.zeros((DT_RANK, N_LAYERS * D_INNER), bf)
    for l in range(N_LAYERS):
        dw[:, l * D_INNER:(l + 1) * D_INNER] = \
            np.asarray(inp["dt_W"], f32)[l].astype(bf)
    out["dtW"] = dw
    ow = np.zeros((128, N_LAYERS * 4 * D_MODEL), bf)
    for l in range(N_LAYERS):
        w = np.asarray(inp["out_proj_W"], f32)[l]  # [512, 256]
        for kb in range(4):
            ow[:, (l * 4 + kb) * D_MODEL:(l * 4 + kb + 1) * D_MODEL] = \
                w[kb * 128:(kb + 1) * 128, :].astype(bf)
    out["outW"] = ow
    lg = np.zeros((128, 2), f32)
    lb = np.zeros((128, 2), f32)
    lg[:, 0] = np.asarray(inp["ln_g"], f32)[:128]
    lg[:, 1] = np.asarray(inp["ln_g"], f32)[128:]
    lb[:, 0] = np.asarray(inp["ln_b"], f32)[:128]
    lb[:, 1] = np.asarray(inp["ln_b"], f32)[128:]
    out["lng"] = lg
    out["lnb"] = lb
    hw = np.asarray(inp["head_W"], f32)  # [16384, 96]
    out["headW"] = np.ascontiguousarray(
        hw.reshape(KHEAD, 128, PRED).transpose(1, 0, 2).reshape(
            128, KHEAD * PRED)).astype(bf)
    out["headb"] = np.broadcast_to(
        np.asarray(inp["head_b"], f32), (NSEQ, PRED)).copy()
    out["ones"] = np.ones((128, 128), f32)
    out["epsc"] = np.full((128, 1), EPS, f32)
    return out


def kernel(**inputs):
    x = np.asarray(inputs["x"], np.float32)          # [16, 7, 512]
    A = -np.exp(np.asarray(inputs["A_log"], np.float64))  # [2, 512, 16]
    # A is d-independent by construction; bake per-(l,s) scales as immediates
    a_scales = tuple(tuple(float(A[l, 0, s]) for s in range(D_STATE))
                     for l in range(N_LAYERS))

    key = a_scales
    if key not in _CACHE:
        _CACHE[key] = _build(a_scales)
    nc = _CACHE[key]

    shared = _prep_shared(inputs)
    xf = x.reshape(B * M, SEQ)
    xpad = np.concatenate([xf, np.repeat(xf[:, -1:], STRIDE, axis=1)], axis=1)
    idx = np.arange(NPATCH)[:, None] * STRIDE + np.arange(PATCH)[None, :]
    allpatch = xpad[:, idx]  # [112, 64, 16]

    in_maps = []
    for c in range(NCORES):
        m = dict(shared)
        pc = allpatch[c * NSEQ:(c + 1) * NSEQ]          # [14, 64, 16]
        m["xpatch"] = np.ascontiguousarray(
            pc.reshape(NT, PATCH).T, np.float32)         # [16, 896]
        in_maps.append(m)

    res = bass_utils.run_bass_kernel_spmd(nc, in_maps, core_ids=list(range(NCORES)))
    global LAST_RESULT
    LAST_RESULT = res
    outs = [res.results[c]["yout"] for c in range(NCORES)]
    y = np.concatenate(outs, axis=0)  # [112, 96]
    return y.reshape(B, M, PRED)


if __name__ == "__main__":
    import reference

    inp = {k: np.asarray(v) for k, v in reference.setup_inputs().items()}
    got = kernel(**inp)
    want = np.asarray(reference.reference(**inp))
    err = np.abs(got - want).max() / (np.abs(want).max() + 1e-30)
    print("Relative error:", err)

